# revision 12
# baseline (speedup 1.0000x reference)
"""Trainium2 Bass kernel for nn_EncodingP (vq_codebook soft-assignment encoding).

Reference computation (B=4, D=256, K=32, H=W=64, N=H*W=4096):
    Xf = X.reshape(B, D, N).transpose(0, 2, 1)            # (B, N, D)
    L[b,n,k] = ||x_bn||^2 - 2 <x_bn, c_k> + ||c_k||^2     # (B, N, K)
    A = softmax(L * scale, axis=-1)                        # (B, N, K)
    E[b,k,d] = sum_n A[b,n,k] * x_bn[d] - (sum_n A[b,n,k]) * c_k[d]

Sharding: 8 cores = 4 batches x 2 halves of N. Each core handles a
(b, N/2) shard of 2048 positions; the host sums the two half-partials per
batch (E is linear in the n-sum).

Per-core dataflow (all fp32):
  phase 1 (PSUM [128,512], col-groups j=0..3 hold n-chunks of 512):
    psL[32j+k, nn] = sum_d -2*C[k,d]*X[d,n] + sum_d Xsq[d,n]   (n = 512j+nn)
  exp: expS = Exp(scale_k * psL + scale_k*c2_k)   (one ACT op, per-partition
       scale/bias vectors; max |scale*L| ~ 70 < 88 so no max-subtraction)
  transpose: 4 PE transposes of expS [128,128] slices -> layout-A tiles
       araw[i, 128c+32j+k] = exp-weight for (n = 512j+128c+i, k)
  normalize: Z = rowsum over each 32-block, A = araw / Z
  phase 2: psE[k, :] += A_tile[n,k]^T @ XTaug_tile[n, 0:257]
       (XTaug has a ones column 256 -> col 256 accumulates Asum[k])
  E = psE[:, 0:256] - Asum * C
"""

import os

import numpy as np

import concourse.bass as bass
import concourse.tile as tile
from concourse import mybir
from concourse.masks import make_identity

B, D, K, H, W = 4, 256, 32, 64, 64
N = H * W            # 4096
NCORES = 8
NSH = B * N // NCORES  # 2048 positions per core
NT = NSH // 128        # 16 n-tiles per core
NAUG = D + 1           # 257: X^T columns + ones column

F32 = mybir.dt.float32

# consts tile column layout
_CT0 = 0      # [0:32)    -2*C^T for d-block 0
_CT1 = 32     # [32:64)   -2*C^T for d-block 1
_ONE = 64     # [64:96)   ones [128, 32]
_SCL = 96     # col 96    tiled scale
_BIA = 97     # col 97    tiled scale*c2
_C32 = 98     # [98:354)  codewords (partitions 0:32)
_CF = _C32 + D  # 354 total const columns


def build_device_kernel(nc):
    xdn_d = nc.declare_dram_parameter("xdn", [D, NSH], F32, isOutput=False)
    xta_d = nc.declare_dram_parameter("xta", [128, NT * NAUG], F32, isOutput=False)
    cst_d = nc.declare_dram_parameter("cst", [128, _CF], F32, isOutput=False)
    out_d = nc.declare_dram_parameter("eout", [K, D], F32, isOutput=True)

    act = mybir.ActivationFunctionType
    alu = mybir.AluOpType

    with tile.TileContext(nc) as tc:
        with (
            tc.tile_pool(name="sb", bufs=1) as sb,
            tc.tile_pool(name="ps", bufs=1, space="PSUM") as ps,
            tc.tile_pool(name="psT", bufs=4, space="PSUM") as psT,
        ):
            cst = sb.tile([128, _CF], F32)
            nc.sync.dma_start(out=cst[:], in_=cst_d[:])
            ident = sb.tile([128, 128], F32)
            make_identity(nc, ident[:])

            # fp32 Matmult carries at most ONE sync wait in walrus codegen
            # (no separate LDWEIGHTS to hold a second one), and we keep every
            # other engine at <=1 cross-engine wait too.  These tiny "touch"
            # ops absorb DMA/gpsimd completions into each engine's program
            # order so real instructions never need two waits.
            dummy = ps.tile([1, 16], F32, tag="dummy")
            scr = sb.tile([128, 16], F32)
            nc.tensor.matmul(dummy[:], cst[:, 0:1], cst[:, 0:16],
                             start=True, stop=True)
            nc.tensor.matmul(dummy[:], ident[:, 0:1], ident[:, 0:16],
                             start=True, stop=True)
            nc.scalar.copy(out=scr[:, 0:2], in_=cst[:, _SCL:_SCL + 2])
            nc.vector.tensor_copy(scr[:, 2:4], cst[:, 0:2])
            nc.scalar.copy(out=scr[:, 4:6], in_=ident[:, 0:2])

            x = sb.tile([128, 2, NSH], F32)
            xsq = sb.tile([128, 2, NSH], F32)
            for q in range(2):
                for d in range(2):
                    nc.sync.dma_start(
                        out=x[:, d, 1024 * q:1024 * (q + 1)],
                        in_=xdn_d[128 * d:128 * (d + 1), 1024 * q:1024 * (q + 1)],
                    )
            xts = sb.tile([128, NT * NAUG], F32)
            for r in range(4):
                nc.sync.dma_start(
                    out=xts[:, 4 * NAUG * r:4 * NAUG * (r + 1)],
                    in_=xta_d[:, 4 * NAUG * r:4 * NAUG * (r + 1)],
                )

            # squares, alternating ACT / DVE
            for q in range(2):
                for d in range(2):
                    src = x[:, d, 1024 * q:1024 * (q + 1)]
                    dst = xsq[:, d, 1024 * q:1024 * (q + 1)]
                    if (2 * q + d) % 2 == 0:
                        nc.scalar.square(out=dst, in_=src)
                    else:
                        nc.vector.tensor_mul(dst, src, src)

            # phase 1: psL[32j+k, nn] = -2*xc + x2 for n = 512j + nn
            psL = ps.tile([128, 512], F32, tag="psL")
            for j in range(4):
                ops = []
                for d in range(2):
                    ops.append((cst[:, 32 * d:32 * (d + 1)],
                                x[:, d, 512 * j:512 * (j + 1)]))
                    ops.append((cst[:, _ONE:_ONE + 32],
                                xsq[:, d, 512 * j:512 * (j + 1)]))
                for i, (lhsT, rhs) in enumerate(ops):
                    nc.tensor.matmul(
                        psL[32 * j:32 * (j + 1), :], lhsT, rhs,
                        start=(i == 0), stop=(i == len(ops) - 1),
                        tile_position=(0, 32 * j),
                    )

            expS = sb.tile([128, 512], F32)
            nc.scalar.activation(
                out=expS[:], in_=psL[:], func=act.Exp,
                bias=cst[:, _BIA:_BIA + 1], scale=cst[:, _SCL:_SCL + 1],
            )

            # transpose to layout A (4 full 128x128 PE transposes)
            araw = sb.tile([128, 512], F32)
            for c in range(4):
                pt = psT.tile([128, 128], F32, tag="pt")
                nc.tensor.transpose(pt[:], expS[:, 128 * c:128 * (c + 1)], ident[:])
                # all evacs on ACT: a mixed-engine writer set on araw makes
                # Tile emit 2 waits on the downstream reduce (walrus limit 1)
                nc.scalar.copy(out=araw[:, 128 * c:128 * (c + 1)], in_=pt[:])

            # softmax denominator + normalize
            z = sb.tile([128, 16], F32)
            rz = sb.tile([128, 16], F32)
            nc.vector.tensor_reduce(
                z[:], araw[:].rearrange("p (g k) -> p g k", k=K),
                axis=mybir.AxisListType.X, op=alu.add,
            )
            nc.vector.reciprocal(rz[:], z[:])
            anorm = sb.tile([128, 512], F32)
            for t in range(NT):
                c, j = t % 4, t // 4
                col = 128 * c + 32 * j
                zc = 4 * c + j
                nc.vector.tensor_scalar_mul(
                    anorm[:, col:col + 32], araw[:, col:col + 32],
                    rz[:, zc:zc + 1],
                )

            # absorb the xts DMA-chunk completions into PE program order
            # before phase 2 (same one-wait rule as above)
            for r in range(4):
                off = 4 * NAUG * r
                nc.tensor.matmul(dummy[:], xts[:, off:off + 1],
                                 xts[:, off:off + 16], start=True, stop=True)

            # phase 2: psE[k, 0:256] = E1, psE[k, 256] = Asum
            psE = ps.tile([K, NAUG], F32, tag="psE")
            for t in range(NT):
                c, j = t % 4, t // 4
                col = 128 * c + 32 * j
                nc.tensor.matmul(
                    psE[:], anorm[:, col:col + 32],
                    xts[:, NAUG * t:NAUG * (t + 1)],
                    start=(t == 0), stop=(t == NT - 1),
                )

            # E = psE[:, 0:256] - Asum * C, applied as one more accumulating
            # matmul: psE[:, 0:256] += diag(-Asum) @ C
            asum = sb.tile([K, 1], F32)
            nc.scalar.mul(out=asum[:], in_=psE[:, D:D + 1], mul=-1.0)
            diagna = sb.tile([K, K], F32)
            nc.scalar.mul(out=diagna[:], in_=ident[0:K, 0:K], mul=asum[:])
            nc.tensor.matmul(psE[:, 0:D], diagna[:], cst[0:K, _C32:_C32 + D],
                             start=False, stop=False, skip_group_check=True)
            esb = sb.tile([K, D], F32)
            nc.scalar.copy(out=esb[:], in_=psE[:, 0:D])
            nc.scalar.dma_start(out=out_d[:], in_=esb[:])

    return nc


def make_host_inputs(X, codewords, scale):
    """Shard + lay out inputs for the 8 cores. Returns list of in_maps."""
    X = np.ascontiguousarray(X, dtype=np.float32)
    codewords = np.asarray(codewords, dtype=np.float32)
    scale = np.asarray(scale, dtype=np.float32)

    c2 = (codewords.astype(np.float64) ** 2).sum(axis=1)
    cst = np.zeros((128, _CF), dtype=np.float32)
    ctn2 = (-2.0 * codewords.T).astype(np.float32)        # [D, K]
    cst[:, _CT0:_CT0 + K] = ctn2[0:128]
    cst[:, _CT1:_CT1 + K] = ctn2[128:256]
    cst[:, _ONE:_ONE + K] = 1.0
    cst[:, _SCL] = np.tile(scale, 4)
    cst[:, _BIA] = np.tile((scale.astype(np.float64) * c2).astype(np.float32), 4)
    cst[0:K, _C32:_C32 + D] = codewords

    Xr = X.reshape(B, D, N)
    in_maps = []
    for core in range(NCORES):
        b, h = core // 2, core % 2
        xdn = np.ascontiguousarray(Xr[b][:, NSH * h:NSH * (h + 1)])
        xt = np.ascontiguousarray(xdn.T)                  # [NSH, D]
        xta = np.concatenate(
            [xt, np.ones((NSH, 1), dtype=np.float32)], axis=1)  # [NSH, 257]
        xta_dev = np.ascontiguousarray(
            xta.reshape(NT, 128, NAUG).transpose(1, 0, 2).reshape(128, NT * NAUG))
        in_maps.append({"xdn": xdn, "xta": xta_dev, "cst": cst})
    return in_maps


def gather_output(results):
    E = np.empty((B, K, D), dtype=np.float32)
    for b in range(B):
        E[b] = results[2 * b]["eout"] + results[2 * b + 1]["eout"]
    return E


_NC_CACHE = {}


def _get_nc():
    if "nc" not in _NC_CACHE:
        # Bacc (not plain Bass): its compile() runs the TRN2 sync-wait
        # legalization (max 1 wait per instruction) that walrus requires.
        from concourse import bacc
        nc = build_device_kernel(bacc.Bacc(None))
        if not nc.is_finalized():
            nc.finalize()  # Bacc.finalize = compile (wait legalization) + freeze
        _NC_CACHE["nc"] = nc
    return _NC_CACHE["nc"]


def _install_ntff_hook_shim():
    """Fabricate antenv.axon_hooks if the image lacks it (profiling only)."""
    import sys
    import types
    try:
        from antenv.axon_hooks import get_axon_ntff_profile_hook  # noqa: F401
        return
    except ImportError:
        pass
    from trn_agent_boot.trn_boot import _ntff_profile_via_ctypes
    hook = _ntff_profile_via_ctypes("/opt/axon/libaxon_pjrt.so")
    mod = types.ModuleType("antenv.axon_hooks")
    mod._hook = hook
    mod.get_axon_ntff_profile_hook = lambda: mod._hook
    mod.set_axon_ntff_profile_hook = lambda h: setattr(mod, "_hook", h)
    sys.modules["antenv.axon_hooks"] = mod
    import antenv
    antenv.axon_hooks = mod


def kernel(X, codewords, scale):
    from concourse.bass_utils import run_bass_kernel_spmd

    nc = _get_nc()
    in_maps = make_host_inputs(X, codewords, scale)
    trace = bool(int(os.environ.get("VQ_KERNEL_TRACE", "0")))
    kwargs = {}
    if trace:
        try:
            _install_ntff_hook_shim()
            tmpdir = os.environ.get("VQ_KERNEL_TMPDIR")
            if tmpdir:
                os.makedirs(tmpdir, exist_ok=True)
                kwargs["tmpdir"] = tmpdir
        except Exception as e:  # profiling must never break execution
            print(f"ntff hook install failed: {e}")
            trace = False
    res = run_bass_kernel_spmd(nc, in_maps, core_ids=list(range(NCORES)),
                               trace=trace, **kwargs)
    if trace and res.exec_time_ns is not None:
        print(f"HW exec time: {res.exec_time_ns} ns")
    return gather_output(res.results)


# revision 14
# speedup vs baseline: 1.0293x; 1.0293x over previous
"""Trainium2 Bass kernel for nn_EncodingP (vq_codebook soft-assignment encoding).

Reference computation (B=4, D=256, K=32, H=W=64, N=H*W=4096):
    Xf = X.reshape(B, D, N).transpose(0, 2, 1)            # (B, N, D)
    L[b,n,k] = ||x_bn||^2 - 2 <x_bn, c_k> + ||c_k||^2     # (B, N, K)
    A = softmax(L * scale, axis=-1)                        # (B, N, K)
    E[b,k,d] = sum_n A[b,n,k] * x_bn[d] - (sum_n A[b,n,k]) * c_k[d]

Sharding: 8 cores = 4 batches x 2 halves of N; host sums the two
half-partials per batch (E is linear in the n-sum).

Per-core dataflow:
  phase 1 (fp16 matmuls -> fp32 PSUM [128,512]; col-groups j=0..3 are
    n-chunks of 512):  psL[32j+k, nn] = -2*xc + x2
  exp (fp32): expS = Exp(scale_k * psL + scale_k*c2_k)  (one ACT op with
    per-partition scale/bias; max |scale*L| ~ 79 < 88 so no max-subtract)
  transpose: 4 PE transposes of expS [128,128] slices -> layout A
  normalize: Z rowsum per 32-block, anorm = araw * (1/Z) (one broadcast TT)
  phase 2: psE[k, :] += anorm_tile^T @ xts_tile[n, 0:257]  (col 256 of xts
    is ones -> accumulates Asum), then psE[:, 0:256] += diag(-Asum) @ C
"""

import os

import numpy as np

import concourse.bass as bass
import concourse.tile as tile
from concourse import mybir
from concourse.masks import make_identity

B, D, K, H, W = 4, 256, 32, 64, 64
N = H * W            # 4096
NCORES = 8
NSH = B * N // NCORES  # 2048 positions per core
NT = NSH // 128        # 16 n-tiles per core
NAUG = D + 1           # 257: X^T columns + ones column

F32 = mybir.dt.float32
F16 = mybir.dt.float16

PH2_FP16 = bool(int(os.environ.get("VQ_PH2_FP16", "0")))

# cst16 (fp16) column layout
_CT0 = 0      # [0:32)    -2*C^T for d-block 0
_CT1 = 32     # [32:64)   -2*C^T for d-block 1
_ONE = 64     # [64:96)   ones
_C16 = 96     # [96:352)  codewords fp16 (partitions 0:32)
_CF16 = _C16 + D
# cst32 (fp32) column layout
_SCL = 0
_BIA = 1
_C32 = 2
_CF32 = _C32 + D


def build_device_kernel(nc):
    ph2dt = F16 if PH2_FP16 else F32
    xdn_d = nc.declare_dram_parameter("xdn", [D, NSH], F16, isOutput=False)
    xta_d = nc.declare_dram_parameter("xta", [128, NT * NAUG], ph2dt,
                                      isOutput=False)
    c16_d = nc.declare_dram_parameter("cst16", [128, _CF16], F16, isOutput=False)
    c32_d = nc.declare_dram_parameter("cst32", [128, _CF32], F32, isOutput=False)
    out_d = nc.declare_dram_parameter("eout", [K, D], F32, isOutput=True)

    act = mybir.ActivationFunctionType
    alu = mybir.AluOpType

    with tile.TileContext(nc) as tc:
        with (
            tc.tile_pool(name="sb", bufs=1) as sb,
            tc.tile_pool(name="ps", bufs=1, space="PSUM") as ps,
            tc.tile_pool(name="psT", bufs=4, space="PSUM") as psT,
        ):
            cst16 = sb.tile([128, _CF16], F16)
            cst32 = sb.tile([128, _CF32], F32)
            x16 = sb.tile([128, 2, NSH], F16)
            xsq = sb.tile([128, 2, NSH], F16)
            xts = sb.tile([128, NT * NAUG], ph2dt)
            ident = sb.tile([128, 128], F32)

            # DMA issue: gpsimd (SWDGE) frees earliest in the Tile preamble,
            # so it carries the phase-1-critical loads; SP and ACT HWDGE
            # rings carry the rest.  One trigger per tensor chunk (trigger
            # dispatch costs ~0.6-1us each).
            nc.gpsimd.dma_start(out=cst16[:], in_=c16_d[:])
            nc.gpsimd.dma_start(out=x16[:, 0, :], in_=xdn_d[0:128, :])
            nc.gpsimd.dma_start(out=x16[:, 1, :], in_=xdn_d[128:256, :])
            half = 8 * NAUG
            nc.sync.dma_start(out=cst32[:], in_=c32_d[:])
            nc.sync.dma_start(out=xts[:, 0:half], in_=xta_d[:, 0:half])
            nc.scalar.dma_start(out=xts[:, half:2 * half],
                                in_=xta_d[:, half:2 * half])
            make_identity(nc, ident[:])

            # one-wait hygiene: absorb DMA/gpsimd completions into each
            # engine's program order early (fp32 matmuls and several DVE/ACT
            # instruction types can carry only one sync wait in walrus).
            dummy = ps.tile([1, 16], F32, tag="dummy")
            scr = sb.tile([128, 16], F32)
            nc.tensor.matmul(dummy[:], cst16[:, 0:1], cst16[:, 0:16],
                             start=True, stop=True)
            nc.tensor.matmul(dummy[:], ident[:, 0:1], ident[:, 0:16],
                             start=True, stop=True)
            nc.scalar.copy(out=scr[:, 0:2], in_=cst32[:, 0:2])
            nc.scalar.copy(out=scr[:, 4:6], in_=ident[:, 0:2])

            # squares on device: xsq = x16^2 (fp16 out, fp32 internal)
            for d in range(2):
                for q in range(2):
                    src = x16[:, d, 1024 * q:1024 * (q + 1)]
                    dst = xsq[:, d, 1024 * q:1024 * (q + 1)]
                    if q == 0:
                        nc.scalar.square(out=dst, in_=src)
                    else:
                        nc.vector.tensor_mul(dst, src, src)

            # phase 1: psL[32j+k, nn] = -2*xc + x2 for n = 512j + nn
            # (groups must be sequential per PSUM zero-region semantics)
            psL = ps.tile([128, 512], F32, tag="psL")
            for j in range(4):
                for d in range(2):
                    nc.tensor.matmul(
                        psL[32 * j:32 * (j + 1), :],
                        cst16[:, 32 * d:32 * (d + 1)],
                        x16[:, d, 512 * j:512 * (j + 1)],
                        start=(d == 0), stop=False,
                        tile_position=(0, 32 * j),
                    )
                    nc.tensor.matmul(
                        psL[32 * j:32 * (j + 1), :],
                        cst16[:, _ONE:_ONE + 32],
                        xsq[:, d, 512 * j:512 * (j + 1)],
                        start=False, stop=(d == 1),
                        tile_position=(0, 32 * j),
                    )

            expS = sb.tile([128, 512], F32)
            nc.scalar.activation(
                out=expS[:], in_=psL[:], func=act.Exp,
                bias=cst32[:, _BIA:_BIA + 1], scale=cst32[:, _SCL:_SCL + 1],
            )

            # transpose to layout A (4 full 128x128 PE transposes, fp32)
            araw = sb.tile([128, 512], F32)
            for c in range(4):
                pt = psT.tile([128, 128], F32, tag="pt")
                nc.tensor.transpose(pt[:], expS[:, 128 * c:128 * (c + 1)], ident[:])
                nc.scalar.copy(out=araw[:, 128 * c:128 * (c + 1)], in_=pt[:])

            # softmax denominator + normalize (single broadcast TT)
            z = sb.tile([128, 16], F32)
            rz = sb.tile([128, 16], F32)
            nc.vector.tensor_reduce(
                z[:], araw[:].rearrange("p (g k) -> p g k", k=K),
                axis=mybir.AxisListType.X, op=alu.add,
            )
            nc.vector.reciprocal(rz[:], z[:])
            anorm = sb.tile([128, 512], ph2dt)
            nc.vector.tensor_tensor(
                out=anorm[:].rearrange("p (g k) -> p g k", k=K),
                in0=araw[:].rearrange("p (g k) -> p g k", k=K),
                in1=rz[:].rearrange("p (g x) -> p g x", x=1).broadcast_to(
                    [128, NT, K]),
                op=alu.mult,
            )

            # absorb the xts DMA completions into PE program order
            nc.tensor.matmul(dummy[:], xts[:, 0:1], xts[:, 0:16],
                             start=True, stop=True)
            nc.tensor.matmul(dummy[:], xts[:, half:half + 1],
                             xts[:, half:half + 16], start=True, stop=True)
            if not PH2_FP16:
                nc.tensor.matmul(dummy[:], cst32[:, _C32:_C32 + 1],
                                 cst32[:, _C32:_C32 + 16],
                                 start=True, stop=True)

            # phase 2: psE[k, 0:256] = E1, psE[k, 256] = Asum
            psE = ps.tile([K, NAUG], F32, tag="psE")
            for t in range(NT):
                c, j = t % 4, t // 4
                col = 128 * c + 32 * j
                nc.tensor.matmul(
                    psE[:], anorm[:, col:col + 32],
                    xts[:, NAUG * t:NAUG * (t + 1)],
                    start=(t == 0), stop=(t == NT - 1),
                )

            # E = psE[:, 0:256] - Asum * C via one more accumulating matmul
            asum = sb.tile([K, 1], F32)
            nc.scalar.mul(out=asum[:], in_=psE[:, D:D + 1], mul=-1.0)
            diagna = sb.tile([K, K], ph2dt)
            nc.scalar.mul(out=diagna[:], in_=ident[0:K, 0:K], mul=asum[:])
            cmat = cst16[0:K, _C16:_C16 + D] if PH2_FP16 \
                else cst32[0:K, _C32:_C32 + D]
            nc.tensor.matmul(psE[:, 0:D], diagna[:], cmat,
                             start=False, stop=False, skip_group_check=True)
            esb = sb.tile([K, D], F32)
            nc.scalar.copy(out=esb[:], in_=psE[:, 0:D])
            nc.scalar.dma_start(out=out_d[:], in_=esb[:])

    return nc


def make_host_inputs(X, codewords, scale):
    """Shard + lay out inputs for the 8 cores. Returns list of in_maps."""
    X = np.ascontiguousarray(X, dtype=np.float32)
    codewords = np.asarray(codewords, dtype=np.float32)
    scale = np.asarray(scale, dtype=np.float32)
    ph2np = np.float16 if PH2_FP16 else np.float32

    c2 = (codewords.astype(np.float64) ** 2).sum(axis=1)
    cst16 = np.zeros((128, _CF16), dtype=np.float16)
    ctn2 = (-2.0 * codewords.T).astype(np.float16)        # [D, K]
    cst16[:, _CT0:_CT0 + K] = ctn2[0:128]
    cst16[:, _CT1:_CT1 + K] = ctn2[128:256]
    cst16[:, _ONE:_ONE + K] = 1.0
    cst16[0:K, _C16:_C16 + D] = codewords.astype(np.float16)
    cst32 = np.zeros((128, _CF32), dtype=np.float32)
    cst32[:, _SCL] = np.tile(scale, 4)
    cst32[:, _BIA] = np.tile((scale.astype(np.float64) * c2).astype(np.float32), 4)
    cst32[0:K, _C32:_C32 + D] = codewords

    Xr = X.reshape(B, D, N)
    in_maps = []
    for core in range(NCORES):
        b, h = core // 2, core % 2
        xdn = np.ascontiguousarray(Xr[b][:, NSH * h:NSH * (h + 1)])
        xdn16 = xdn.astype(np.float16)
        xt = np.ascontiguousarray(xdn.T)                  # [NSH, D] fp32
        xta = np.concatenate(
            [xt, np.ones((NSH, 1), dtype=np.float32)], axis=1).astype(ph2np)
        xta_dev = np.ascontiguousarray(
            xta.reshape(NT, 128, NAUG).transpose(1, 0, 2).reshape(128, NT * NAUG))
        in_maps.append({"xdn": xdn16, "xta": xta_dev,
                        "cst16": cst16, "cst32": cst32})
    return in_maps


def gather_output(results):
    E = np.empty((B, K, D), dtype=np.float32)
    for b in range(B):
        E[b] = results[2 * b]["eout"] + results[2 * b + 1]["eout"]
    return E


_NC_CACHE = {}


def _get_nc():
    if "nc" not in _NC_CACHE:
        # Bacc (not plain Bass): its compile() runs the TRN2 sync-wait
        # legalization (max 1 wait per instruction) that walrus requires.
        from concourse import bacc
        nc = build_device_kernel(bacc.Bacc(None))
        if not nc.is_finalized():
            nc.finalize()  # Bacc.finalize = compile (wait legalization) + freeze
        _NC_CACHE["nc"] = nc
    return _NC_CACHE["nc"]


def _install_ntff_hook_shim():
    """Fabricate antenv.axon_hooks if the image lacks it (profiling only)."""
    import sys
    import types
    try:
        from antenv.axon_hooks import get_axon_ntff_profile_hook  # noqa: F401
        return
    except ImportError:
        pass
    from trn_agent_boot.trn_boot import _ntff_profile_via_ctypes
    hook = _ntff_profile_via_ctypes("/opt/axon/libaxon_pjrt.so")
    mod = types.ModuleType("antenv.axon_hooks")
    mod._hook = hook
    mod.get_axon_ntff_profile_hook = lambda: mod._hook
    mod.set_axon_ntff_profile_hook = lambda h: setattr(mod, "_hook", h)
    sys.modules["antenv.axon_hooks"] = mod
    import antenv
    antenv.axon_hooks = mod


def kernel(X, codewords, scale):
    from concourse.bass_utils import run_bass_kernel_spmd

    nc = _get_nc()
    in_maps = make_host_inputs(X, codewords, scale)
    trace = bool(int(os.environ.get("VQ_KERNEL_TRACE", "0")))
    kwargs = {}
    if trace:
        try:
            _install_ntff_hook_shim()
            tmpdir = os.environ.get("VQ_KERNEL_TMPDIR")
            if tmpdir:
                os.makedirs(tmpdir, exist_ok=True)
                kwargs["tmpdir"] = tmpdir
        except Exception as e:  # profiling must never break execution
            print(f"ntff hook install failed: {e}")
            trace = False
    res = run_bass_kernel_spmd(nc, in_maps, core_ids=list(range(NCORES)),
                               trace=trace, **kwargs)
    if trace and res.exec_time_ns is not None:
        print(f"HW exec time: {res.exec_time_ns} ns")
    return gather_output(res.results)


# revision 18
# speedup vs baseline: 1.1876x; 1.1538x over previous
"""Trainium2 Bass kernel for nn_EncodingP (vq_codebook soft-assignment encoding).

Reference computation (B=4, D=256, K=32, H=W=64, N=H*W=4096):
    Xf = X.reshape(B, D, N).transpose(0, 2, 1)            # (B, N, D)
    L[b,n,k] = ||x_bn||^2 - 2 <x_bn, c_k> + ||c_k||^2     # (B, N, K)
    A = softmax(L * scale, axis=-1)                        # (B, N, K)
    E[b,k,d] = sum_n A[b,n,k] * x_bn[d] - (sum_n A[b,n,k]) * c_k[d]

Sharding: 8 cores = 4 batches x 2 halves of N; host sums the two
half-partials per batch (E is linear in the n-sum).

Per-core dataflow:
  phase 1 (fp16 matmuls -> fp32 PSUM [128,512]; col-group j holds n-chunk j):
    psL[32j+k, nn] = -2*xc + x2     (x2 via an all-ones stationary over x^2)
  exp (fp32): expS = Exp(scale_k * psL + scale_k*c2_k)  (one ACT op with
    per-partition scale/bias; max |scale*L| ~ 79 < 88 so no max-subtract)
  transpose: 4 PE transposes of expS [128,128] slices -> layout A (araw)
  normalize per 128-col block: Z rowsum per 32-block, anorm = araw * (1/Z)
  phase 2 (4-way col-tiled): psE4[32g+k, :] += anorm_t^T @ xts_t  for the
    4 tiles t = 4g + c of group g (xts col 256 is ones -> Asum partials),
    then psE = stacked_I32^T @ psE4_evac (combine) + diag(-Asum) @ C
"""

import os

import numpy as np

import concourse.bass as bass
import concourse.tile as tile
from concourse import mybir
from concourse.masks import make_identity

B, D, K, H, W = 4, 256, 32, 64, 64
N = H * W            # 4096
NCORES = 8
NSH = B * N // NCORES  # 2048 positions per core
NT = NSH // 128        # 16 n-tiles per core
NAUG = D + 1           # 257: X^T columns + ones column

F32 = mybir.dt.float32
F16 = mybir.dt.float16

PH2_FP16 = bool(int(os.environ.get("VQ_PH2_FP16", "0")))

# cst16 (fp16) column layout
_CT0 = 0      # [0:32)    -2*C^T for d-block 0
_CT1 = 32     # [32:64)   -2*C^T for d-block 1
_ONE = 64     # [64:96)   ones
_C16 = 96     # [96:352)  codewords fp16 (partitions 0:32)
_S16 = 96 + D  # [352:384) stacked I32 (fp16)
_CF16 = _S16 + K
# cst32 (fp32) column layout
_SCL = 0
_BIA = 1
_C32 = 2           # [2:258)  codewords fp32 (partitions 0:32)
_S32 = 2 + D       # [258:290) stacked I32 (fp32): rows 32g+k, col k = 1
_CF32 = _S32 + K


def build_device_kernel(nc):
    ph2dt = F16 if PH2_FP16 else F32
    xdn_d = nc.declare_dram_parameter("xdn", [D, NSH], F16, isOutput=False)
    xta_d = nc.declare_dram_parameter("xta", [128, NT * NAUG], ph2dt,
                                      isOutput=False)
    c16_d = nc.declare_dram_parameter("cst16", [128, _CF16], F16, isOutput=False)
    c32_d = nc.declare_dram_parameter("cst32", [128, _CF32], F32, isOutput=False)
    out_d = nc.declare_dram_parameter("eout", [K, D], F32, isOutput=True)

    act = mybir.ActivationFunctionType
    alu = mybir.AluOpType

    with tile.TileContext(nc) as tc:
        with (
            tc.tile_pool(name="sb", bufs=1) as sb,
            tc.tile_pool(name="ps", bufs=1, space="PSUM") as ps,
            tc.tile_pool(name="psT", bufs=4, space="PSUM") as psT,
        ):
            cst16 = sb.tile([128, _CF16], F16)
            cst32 = sb.tile([128, _CF32], F32)
            x16 = sb.tile([128, 2, NSH], F16)
            xsq = sb.tile([128, 2, NSH], F16)
            xts = sb.tile([128, NT * NAUG], ph2dt)
            ident = sb.tile([128, 128], F32)

            # DMA issue: phase-1-critical tensors on the SP HWDGE ring,
            # the rest on the ACT ring.  One trigger per tensor chunk
            # (trigger dispatch costs ~0.6-0.8us each on the issuing queue).
            nc.sync.dma_start(out=cst16[:], in_=c16_d[:])
            nc.sync.dma_start(out=x16[:, 0, :], in_=xdn_d[0:128, :])
            nc.sync.dma_start(out=x16[:, 1, :], in_=xdn_d[128:256, :])
            half = 8 * NAUG
            nc.scalar.dma_start(out=cst32[:], in_=c32_d[:])
            nc.scalar.dma_start(out=xts[:, 0:half], in_=xta_d[:, 0:half])
            nc.scalar.dma_start(out=xts[:, half:2 * half],
                                in_=xta_d[:, half:2 * half])
            make_identity(nc, ident[:])

            # one-wait hygiene: absorb DMA/gpsimd completions into each
            # engine's program order early (several instruction types can
            # carry only one sync wait; extra waits cost EVSEM chains).
            dummy = ps.tile([1, 16], F32, tag="dummy")
            scr = sb.tile([128, 16], F32)
            nc.tensor.matmul(dummy[:], cst16[:, 0:1], cst16[:, 0:16],
                             start=True, stop=True)
            nc.tensor.matmul(dummy[:], ident[:, 0:1], ident[:, 0:16],
                             start=True, stop=True)
            nc.scalar.copy(out=scr[:, 0:2], in_=cst32[:, 0:2])
            nc.scalar.copy(out=scr[:, 4:6], in_=ident[:, 0:2])
            nc.vector.tensor_copy(scr[:, 6:8], cst32[:, 0:2])

            # squares on device: xsq = x16^2 (fp16 out, fp32 internal);
            # the two chunks of each d-block go to different engines so a
            # d-block's squares finish in one op-latency
            for d in range(2):
                for q in range(2):
                    src = x16[:, d, 1024 * q:1024 * (q + 1)]
                    dst = xsq[:, d, 1024 * q:1024 * (q + 1)]
                    if q == 0:
                        nc.scalar.square(out=dst, in_=src)
                    else:
                        nc.vector.tensor_mul(dst, src, src)

            # phase 1: psL[32j+k, nn] = -2*xc + x2 for n = 512j + nn.
            # d-outer so all d0 matmuls can run before the d1 DMA lands;
            # interleaved starts across partition-disjoint col groups are
            # numerically fine (per-partition pending-zero), only the sim's
            # partition-blind group check needs skipping.
            psL = ps.tile([128, 512], F32, tag="psL")
            for d in range(2):
                for j in range(4):
                    nc.tensor.matmul(
                        psL[32 * j:32 * (j + 1), :],
                        cst16[:, 32 * d:32 * (d + 1)],
                        x16[:, d, 512 * j:512 * (j + 1)],
                        start=(d == 0), stop=False,
                        tile_position=(0, 32 * j), skip_group_check=True,
                    )
                    nc.tensor.matmul(
                        psL[32 * j:32 * (j + 1), :],
                        cst16[:, _ONE:_ONE + 32],
                        xsq[:, d, 512 * j:512 * (j + 1)],
                        start=False, stop=(d == 1),
                        tile_position=(0, 32 * j), skip_group_check=True,
                    )

            expS = sb.tile([128, 512], F32)
            nc.scalar.activation(
                out=expS[:], in_=psL[:], func=act.Exp,
                bias=cst32[:, _BIA:_BIA + 1], scale=cst32[:, _SCL:_SCL + 1],
            )

            # transpose to layout A + per-block softmax normalization,
            # pipelined per 128-col block c
            araw = sb.tile([128, 512], F32)
            z = sb.tile([128, 16], F32)
            rz = sb.tile([128, 16], F32)
            anorm = sb.tile([128, 512], ph2dt)
            for c in range(4):
                pt = psT.tile([128, 128], F32, tag="pt")
                nc.tensor.transpose(pt[:], expS[:, 128 * c:128 * (c + 1)], ident[:])
                blk = slice(128 * c, 128 * (c + 1))
                if c % 2 == 0:
                    nc.scalar.copy(out=araw[:, blk], in_=pt[:])
                else:
                    nc.vector.tensor_copy(araw[:, blk], pt[:])
                zc = slice(4 * c, 4 * (c + 1))
                nc.vector.tensor_reduce(
                    z[:, zc], araw[:, blk].rearrange("p (g k) -> p g k", k=K),
                    axis=mybir.AxisListType.X, op=alu.add,
                )
                nc.vector.reciprocal(rz[:, zc], z[:, zc])
                nc.vector.tensor_tensor(
                    out=anorm[:, blk].rearrange("p (g k) -> p g k", k=K),
                    in0=araw[:, blk].rearrange("p (g k) -> p g k", k=K),
                    in1=rz[:, zc].rearrange("p (g x) -> p g x", x=1).broadcast_to(
                        [128, 4, K]),
                    op=alu.mult,
                )

            # absorb the xts DMA completions into PE program order
            nc.tensor.matmul(dummy[:], xts[:, 0:1], xts[:, 0:16],
                             start=True, stop=True)
            nc.tensor.matmul(dummy[:], xts[:, half:half + 1],
                             xts[:, half:half + 16], start=True, stop=True)
            if not PH2_FP16:
                nc.tensor.matmul(dummy[:], cst32[:, _C32:_C32 + 1],
                                 cst32[:, _C32:_C32 + 16],
                                 start=True, stop=True)

            # phase 2, 4-way col-tiled: group g accumulates tiles t = 4g + c
            # into psE4[32g:32g+32, :]; c-major order so the 4 groups run
            # concurrently in disjoint 32-col array strips
            # free width 272 (not 257): 32-partition slice offsets must be
            # 2048-byte aligned for PSUM pending-zero bookkeeping
            psE4 = ps.tile([128, 272], F32, tag="psE4")
            for c in range(4):
                for g in range(4):
                    t = 4 * g + c
                    col = 128 * c + 32 * g
                    nc.tensor.matmul(
                        psE4[32 * g:32 * (g + 1), 0:NAUG],
                        anorm[:, col:col + 32],
                        xts[:, NAUG * t:NAUG * (t + 1)],
                        start=(c == 0), stop=(c == 3),
                        tile_position=(0, 32 * g), skip_group_check=True,
                    )

            # combine the 4 partial groups: psE = stacked_I32^T @ psE4
            full4 = sb.tile([128, NAUG], F32)
            nc.scalar.copy(out=full4[:], in_=psE4[:, 0:NAUG])
            psE = ps.tile([K, NAUG], F32, tag="psE")
            stk = cst16[:, _S16:_S16 + K] if PH2_FP16 \
                else cst32[:, _S32:_S32 + K]
            stk_rhs = full4[:]
            if PH2_FP16:
                # matmul operands must share dtype; keep combine in fp32
                stk = cst32[:, _S32:_S32 + K]
            nc.tensor.matmul(psE[:], stk, stk_rhs, start=True, stop=True)

            # E = psE[:, 0:256] - Asum * C via one more accumulating matmul
            asum = sb.tile([K, 1], F32)
            nc.scalar.mul(out=asum[:], in_=psE[:, D:D + 1], mul=-1.0)
            diagna = sb.tile([K, K], F32)
            nc.scalar.mul(out=diagna[:], in_=ident[0:K, 0:K], mul=asum[:])
            nc.tensor.matmul(psE[:, 0:D], diagna[:], cst32[0:K, _C32:_C32 + D],
                             start=False, stop=False, skip_group_check=True)
            esb = sb.tile([K, D], F32)
            nc.scalar.copy(out=esb[:], in_=psE[:, 0:D])
            nc.scalar.dma_start(out=out_d[:], in_=esb[:])

    return nc


def make_host_inputs(X, codewords, scale):
    """Shard + lay out inputs for the 8 cores. Returns list of in_maps."""
    X = np.ascontiguousarray(X, dtype=np.float32)
    codewords = np.asarray(codewords, dtype=np.float32)
    scale = np.asarray(scale, dtype=np.float32)
    ph2np = np.float16 if PH2_FP16 else np.float32

    stacked = np.zeros((128, K), dtype=np.float32)
    for g in range(4):
        stacked[32 * g:32 * (g + 1)] = np.eye(K, dtype=np.float32)

    c2 = (codewords.astype(np.float64) ** 2).sum(axis=1)
    cst16 = np.zeros((128, _CF16), dtype=np.float16)
    ctn2 = (-2.0 * codewords.T).astype(np.float16)        # [D, K]
    cst16[:, _CT0:_CT0 + K] = ctn2[0:128]
    cst16[:, _CT1:_CT1 + K] = ctn2[128:256]
    cst16[:, _ONE:_ONE + K] = 1.0
    cst16[0:K, _C16:_C16 + D] = codewords.astype(np.float16)
    cst16[:, _S16:_S16 + K] = stacked.astype(np.float16)
    cst32 = np.zeros((128, _CF32), dtype=np.float32)
    cst32[:, _SCL] = np.tile(scale, 4)
    cst32[:, _BIA] = np.tile((scale.astype(np.float64) * c2).astype(np.float32), 4)
    cst32[0:K, _C32:_C32 + D] = codewords
    cst32[:, _S32:_S32 + K] = stacked

    Xr = X.reshape(B, D, N)
    in_maps = []
    for core in range(NCORES):
        b, h = core // 2, core % 2
        xdn = np.ascontiguousarray(Xr[b][:, NSH * h:NSH * (h + 1)])
        xdn16 = xdn.astype(np.float16)
        xt = np.ascontiguousarray(xdn.T)                  # [NSH, D] fp32
        xta = np.concatenate(
            [xt, np.ones((NSH, 1), dtype=np.float32)], axis=1).astype(ph2np)
        xta_dev = np.ascontiguousarray(
            xta.reshape(NT, 128, NAUG).transpose(1, 0, 2).reshape(128, NT * NAUG))
        in_maps.append({"xdn": xdn16, "xta": xta_dev,
                        "cst16": cst16, "cst32": cst32})
    return in_maps


def gather_output(results):
    E = np.empty((B, K, D), dtype=np.float32)
    for b in range(B):
        E[b] = results[2 * b]["eout"] + results[2 * b + 1]["eout"]
    return E


_NC_CACHE = {}


def _get_nc():
    if "nc" not in _NC_CACHE:
        # Bacc (not plain Bass): its compile() runs the TRN2 sync-wait
        # legalization (max 1 wait per instruction) that walrus requires.
        from concourse import bacc
        nc = build_device_kernel(bacc.Bacc(None))
        if not nc.is_finalized():
            nc.finalize()  # Bacc.finalize = compile (wait legalization) + freeze
        _NC_CACHE["nc"] = nc
    return _NC_CACHE["nc"]


def _install_ntff_hook_shim():
    """Fabricate antenv.axon_hooks if the image lacks it (profiling only)."""
    import sys
    import types
    try:
        from antenv.axon_hooks import get_axon_ntff_profile_hook  # noqa: F401
        return
    except ImportError:
        pass
    from trn_agent_boot.trn_boot import _ntff_profile_via_ctypes
    hook = _ntff_profile_via_ctypes("/opt/axon/libaxon_pjrt.so")
    mod = types.ModuleType("antenv.axon_hooks")
    mod._hook = hook
    mod.get_axon_ntff_profile_hook = lambda: mod._hook
    mod.set_axon_ntff_profile_hook = lambda h: setattr(mod, "_hook", h)
    sys.modules["antenv.axon_hooks"] = mod
    import antenv
    antenv.axon_hooks = mod


def kernel(X, codewords, scale):
    from concourse.bass_utils import run_bass_kernel_spmd

    nc = _get_nc()
    in_maps = make_host_inputs(X, codewords, scale)
    trace = bool(int(os.environ.get("VQ_KERNEL_TRACE", "0")))
    kwargs = {}
    if trace:
        try:
            _install_ntff_hook_shim()
            tmpdir = os.environ.get("VQ_KERNEL_TMPDIR")
            if tmpdir:
                os.makedirs(tmpdir, exist_ok=True)
                kwargs["tmpdir"] = tmpdir
        except Exception as e:  # profiling must never break execution
            print(f"ntff hook install failed: {e}")
            trace = False
    res = run_bass_kernel_spmd(nc, in_maps, core_ids=list(range(NCORES)),
                               trace=trace, **kwargs)
    if trace and res.exec_time_ns is not None:
        print(f"HW exec time: {res.exec_time_ns} ns")
    return gather_output(res.results)


# revision 20
# speedup vs baseline: 1.2912x; 1.0872x over previous
"""Trainium2 Bass kernel for nn_EncodingP (vq_codebook soft-assignment encoding).

Reference computation (B=4, D=256, K=32, H=W=64, N=H*W=4096):
    Xf = X.reshape(B, D, N).transpose(0, 2, 1)            # (B, N, D)
    L[b,n,k] = ||x_bn||^2 - 2 <x_bn, c_k> + ||c_k||^2     # (B, N, K)
    A = softmax(L * scale, axis=-1)                        # (B, N, K)
    E[b,k,d] = sum_n A[b,n,k] * x_bn[d] - (sum_n A[b,n,k]) * c_k[d]

Sharding: 8 cores = 4 batches x 2 halves of N; host sums the two
half-partials per batch (E is linear in the n-sum).

Per-core dataflow:
  phase 1 (fp16 matmuls -> fp32 PSUM [128,512]; col-group j holds n-chunk j):
    psL[32j+k, nn] = -2*xc + x2     (x2 via an all-ones stationary over x^2)
  exp (fp32): expS = Exp(scale_k * psL + scale_k*c2_k)  (one ACT op with
    per-partition scale/bias; max |scale*L| ~ 79 < 88 so no max-subtract)
  transpose: 4 PE transposes of expS [128,128] slices -> layout A (araw)
  normalize per 128-col block: Z rowsum per 32-block, anorm = araw * (1/Z)
  phase 2 (4-way col-tiled): psE4[32g+k, :] += anorm_t^T @ xts_t  for the
    4 tiles t = 4g + c of group g (xts col 256 is ones -> Asum partials),
    then psE = stacked_I32^T @ psE4_evac (combine) + diag(-Asum) @ C
"""

import os

import numpy as np

import concourse.bass as bass
import concourse.tile as tile
from concourse import mybir
from concourse.masks import make_identity

B, D, K, H, W = 4, 256, 32, 64, 64
N = H * W            # 4096
NCORES = 8
NSH = B * N // NCORES  # 2048 positions per core
NT = NSH // 128        # 16 n-tiles per core
NAUG = D + 1           # 257: X^T columns + ones column

F32 = mybir.dt.float32
F16 = mybir.dt.float16

PH2_FP16 = bool(int(os.environ.get("VQ_PH2_FP16", "0")))

# cst16 (fp16) column layout
_CT0 = 0      # [0:32)    -2*C^T for d-block 0
_CT1 = 32     # [32:64)   -2*C^T for d-block 1
_ONE = 64     # [64:96)   ones
_C16 = 96     # [96:352)  codewords fp16 (partitions 0:32)
_S16 = 96 + D  # [352:384) stacked I32 (fp16)
_CF16 = _S16 + K
# cst32 (fp32) column layout
_SCL = 0
_BIA = 1
_C32 = 2           # [2:258)  codewords fp32 (partitions 0:32)
_S32 = 2 + D       # [258:290) stacked I32 (fp32): rows 32g+k, col k = 1
_CF32 = _S32 + K


def build_device_kernel(nc):
    ph2dt = F16 if PH2_FP16 else F32
    xdn_d = nc.declare_dram_parameter("xdn", [D, NSH], F16, isOutput=False)
    xta_d = nc.declare_dram_parameter("xta", [128, NT * NAUG], ph2dt,
                                      isOutput=False)
    c16_d = nc.declare_dram_parameter("cst16", [128, _CF16], F16, isOutput=False)
    c32_d = nc.declare_dram_parameter("cst32", [128, _CF32], F32, isOutput=False)
    out_d = nc.declare_dram_parameter("eout", [K, D], F32, isOutput=True)

    act = mybir.ActivationFunctionType
    alu = mybir.AluOpType

    with tile.TileContext(nc) as tc:
        with (
            tc.tile_pool(name="sb", bufs=1) as sb,
            tc.tile_pool(name="ps", bufs=1, space="PSUM") as ps,
            tc.tile_pool(name="psT", bufs=4, space="PSUM") as psT,
        ):
            cst16 = sb.tile([128, _CF16], F16)
            cst32 = sb.tile([128, _CF32], F32)
            x0 = sb.tile([128, NSH], F16)
            x1 = sb.tile([128, NSH], F16)
            sq0 = sb.tile([128, NSH], F16)
            sq1 = sb.tile([128, NSH], F16)
            xts = sb.tile([128, NT * NAUG], ph2dt)
            ident = sb.tile([128, 128], F32)

            # DMA issue: phase-1-critical tensors on the SP HWDGE ring,
            # the rest on the ACT ring.  One trigger per tensor chunk
            # (trigger dispatch costs ~0.6-0.8us each on the issuing queue).
            nc.sync.dma_start(out=cst16[:], in_=c16_d[:])
            nc.sync.dma_start(out=x0[:], in_=xdn_d[0:128, :])
            nc.sync.dma_start(out=x1[:], in_=xdn_d[128:256, :])
            half = 8 * NAUG
            nc.scalar.dma_start(out=cst32[:], in_=c32_d[:])
            # xts triggers are issued later in ACT program order (after the
            # first square) so the x16 stream gets full HBM bandwidth first
            make_identity(nc, ident[:])

            # one-wait hygiene: absorb DMA/gpsimd completions into each
            # engine's program order early (several instruction types can
            # carry only one sync wait; extra waits cost EVSEM chains).
            dummy = ps.tile([1, 128], F32, tag="dummy")
            scr = sb.tile([128, 16], F32)
            nc.tensor.matmul(dummy[:, 0:16], cst16[:, 0:1], cst16[:, 0:16],
                             start=True, stop=True)
            # HAM warmup: ~4us of fp32 dummy matmuls on the identity while
            # the x16 DMA streams, so phase 1 runs at 2.4 GHz instead of 1.2
            for _ in range(8):
                nc.tensor.matmul(dummy[:], ident[:, 0:1], ident[:],
                                 start=True, stop=True)
            nc.scalar.copy(out=scr[:, 0:2], in_=cst32[:, 0:2])
            nc.scalar.copy(out=scr[:, 4:6], in_=ident[:, 0:2])
            nc.vector.tensor_copy(scr[:, 6:8], cst32[:, 0:2])

            # squares on device: sq = x^2 (fp16 out, fp32 internal); the two
            # chunks of each d-block go to different engines so a d-block's
            # squares finish in one op-latency.  The xts DMA triggers ride
            # the ACT queue between squares: they fire only once x0 has
            # landed, keeping early HBM bandwidth on the critical x stream.
            nc.scalar.square(out=sq0[:, 0:1024], in_=x0[:, 0:1024])
            nc.vector.tensor_mul(sq0[:, 1024:2048], x0[:, 1024:2048],
                                 x0[:, 1024:2048])
            nc.scalar.dma_start(out=xts[:, 0:half], in_=xta_d[:, 0:half])
            nc.scalar.square(out=sq1[:, 0:1024], in_=x1[:, 0:1024])
            nc.vector.tensor_mul(sq1[:, 1024:2048], x1[:, 1024:2048],
                                 x1[:, 1024:2048])
            nc.scalar.dma_start(out=xts[:, half:2 * half],
                                in_=xta_d[:, half:2 * half])

            # phase 1: psL[32j+k, nn] = -2*xc + x2 for n = 512j + nn.
            # d-outer so all d0 matmuls can run before the d1 DMA lands;
            # interleaved starts across partition-disjoint col groups are
            # numerically fine (per-partition pending-zero), only the sim's
            # partition-blind group check needs skipping.
            psL = ps.tile([128, 512], F32, tag="psL")
            for d, xt_ in ((0, x0), (1, x1)):
                for j in range(4):
                    nc.tensor.matmul(
                        psL[32 * j:32 * (j + 1), :],
                        cst16[:, 32 * d:32 * (d + 1)],
                        xt_[:, 512 * j:512 * (j + 1)],
                        start=(d == 0), stop=False,
                        tile_position=(0, 32 * j), skip_group_check=True,
                    )
            for d, sq_ in ((0, sq0), (1, sq1)):
                for j in range(4):
                    nc.tensor.matmul(
                        psL[32 * j:32 * (j + 1), :],
                        cst16[:, _ONE:_ONE + 32],
                        sq_[:, 512 * j:512 * (j + 1)],
                        start=False, stop=(d == 1),
                        tile_position=(0, 32 * j), skip_group_check=True,
                    )

            expS = sb.tile([128, 512], F32)
            nc.scalar.activation(
                out=expS[:], in_=psL[:], func=act.Exp,
                bias=cst32[:, _BIA:_BIA + 1], scale=cst32[:, _SCL:_SCL + 1],
            )

            # transpose to layout A + per-block softmax normalization,
            # pipelined per 128-col block c
            araw = sb.tile([128, 512], F32)
            z = sb.tile([128, 16], F32)
            rz = sb.tile([128, 16], F32)
            anorm = sb.tile([128, 512], ph2dt)
            for c in range(4):
                pt = psT.tile([128, 128], F32, tag="pt")
                nc.tensor.transpose(pt[:], expS[:, 128 * c:128 * (c + 1)], ident[:])
                blk = slice(128 * c, 128 * (c + 1))
                if c % 2 == 0:
                    nc.scalar.copy(out=araw[:, blk], in_=pt[:])
                else:
                    nc.vector.tensor_copy(araw[:, blk], pt[:])
                zc = slice(4 * c, 4 * (c + 1))
                nc.vector.tensor_reduce(
                    z[:, zc], araw[:, blk].rearrange("p (g k) -> p g k", k=K),
                    axis=mybir.AxisListType.X, op=alu.add,
                )
                nc.vector.reciprocal(rz[:, zc], z[:, zc])
                nc.vector.tensor_tensor(
                    out=anorm[:, blk].rearrange("p (g k) -> p g k", k=K),
                    in0=araw[:, blk].rearrange("p (g k) -> p g k", k=K),
                    in1=rz[:, zc].rearrange("p (g x) -> p g x", x=1).broadcast_to(
                        [128, 4, K]),
                    op=alu.mult,
                )

            # absorb the xts DMA completions into PE program order
            nc.tensor.matmul(dummy[:, 0:16], xts[:, 0:1], xts[:, 0:16],
                             start=True, stop=True)
            nc.tensor.matmul(dummy[:, 0:16], xts[:, half:half + 1],
                             xts[:, half:half + 16], start=True, stop=True)
            if not PH2_FP16:
                nc.tensor.matmul(dummy[:, 0:16], cst32[:, _C32:_C32 + 1],
                                 cst32[:, _C32:_C32 + 16],
                                 start=True, stop=True)

            # phase 2, 4-way col-tiled: group g accumulates tiles t = 4g + c
            # into psE4[32g:32g+32, :]; c-major order so the 4 groups run
            # concurrently in disjoint 32-col array strips
            # free width 272 (not 257): 32-partition slice offsets must be
            # 2048-byte aligned for PSUM pending-zero bookkeeping
            psE4 = ps.tile([128, 272], F32, tag="psE4")
            for c in range(4):
                for g in range(4):
                    t = 4 * g + c
                    col = 128 * c + 32 * g
                    nc.tensor.matmul(
                        psE4[32 * g:32 * (g + 1), 0:NAUG],
                        anorm[:, col:col + 32],
                        xts[:, NAUG * t:NAUG * (t + 1)],
                        start=(c == 0), stop=(c == 3),
                        tile_position=(0, 32 * g), skip_group_check=True,
                    )

            # combine the 4 partial groups: psE = stacked_I32^T @ psE4
            full4 = sb.tile([128, NAUG], F32)
            nc.scalar.copy(out=full4[:], in_=psE4[:, 0:NAUG])
            psE = ps.tile([K, NAUG], F32, tag="psE")
            stk = cst16[:, _S16:_S16 + K] if PH2_FP16 \
                else cst32[:, _S32:_S32 + K]
            stk_rhs = full4[:]
            if PH2_FP16:
                # matmul operands must share dtype; keep combine in fp32
                stk = cst32[:, _S32:_S32 + K]
            nc.tensor.matmul(psE[:], stk, stk_rhs, start=True, stop=True)

            # E = psE[:, 0:256] - Asum * C via one more accumulating matmul
            asum = sb.tile([K, 1], F32)
            nc.scalar.mul(out=asum[:], in_=psE[:, D:D + 1], mul=-1.0)
            diagna = sb.tile([K, K], F32)
            nc.scalar.mul(out=diagna[:], in_=ident[0:K, 0:K], mul=asum[:])
            nc.tensor.matmul(psE[:, 0:D], diagna[:], cst32[0:K, _C32:_C32 + D],
                             start=False, stop=False, skip_group_check=True)
            esb = sb.tile([K, D], F32)
            nc.scalar.copy(out=esb[:], in_=psE[:, 0:D])
            nc.scalar.dma_start(out=out_d[:], in_=esb[:])

    return nc


def make_host_inputs(X, codewords, scale):
    """Shard + lay out inputs for the 8 cores. Returns list of in_maps."""
    X = np.ascontiguousarray(X, dtype=np.float32)
    codewords = np.asarray(codewords, dtype=np.float32)
    scale = np.asarray(scale, dtype=np.float32)
    ph2np = np.float16 if PH2_FP16 else np.float32

    stacked = np.zeros((128, K), dtype=np.float32)
    for g in range(4):
        stacked[32 * g:32 * (g + 1)] = np.eye(K, dtype=np.float32)

    c2 = (codewords.astype(np.float64) ** 2).sum(axis=1)
    cst16 = np.zeros((128, _CF16), dtype=np.float16)
    ctn2 = (-2.0 * codewords.T).astype(np.float16)        # [D, K]
    cst16[:, _CT0:_CT0 + K] = ctn2[0:128]
    cst16[:, _CT1:_CT1 + K] = ctn2[128:256]
    cst16[:, _ONE:_ONE + K] = 1.0
    cst16[0:K, _C16:_C16 + D] = codewords.astype(np.float16)
    cst16[:, _S16:_S16 + K] = stacked.astype(np.float16)
    cst32 = np.zeros((128, _CF32), dtype=np.float32)
    cst32[:, _SCL] = np.tile(scale, 4)
    cst32[:, _BIA] = np.tile((scale.astype(np.float64) * c2).astype(np.float32), 4)
    cst32[0:K, _C32:_C32 + D] = codewords
    cst32[:, _S32:_S32 + K] = stacked

    Xr = X.reshape(B, D, N)
    in_maps = []
    for core in range(NCORES):
        b, h = core // 2, core % 2
        xdn = np.ascontiguousarray(Xr[b][:, NSH * h:NSH * (h + 1)])
        xdn16 = xdn.astype(np.float16)
        xt = np.ascontiguousarray(xdn.T)                  # [NSH, D] fp32
        xta = np.concatenate(
            [xt, np.ones((NSH, 1), dtype=np.float32)], axis=1).astype(ph2np)
        xta_dev = np.ascontiguousarray(
            xta.reshape(NT, 128, NAUG).transpose(1, 0, 2).reshape(128, NT * NAUG))
        in_maps.append({"xdn": xdn16, "xta": xta_dev,
                        "cst16": cst16, "cst32": cst32})
    return in_maps


def gather_output(results):
    E = np.empty((B, K, D), dtype=np.float32)
    for b in range(B):
        E[b] = results[2 * b]["eout"] + results[2 * b + 1]["eout"]
    return E


_NC_CACHE = {}


def _get_nc():
    if "nc" not in _NC_CACHE:
        # Bacc (not plain Bass): its compile() runs the TRN2 sync-wait
        # legalization (max 1 wait per instruction) that walrus requires.
        from concourse import bacc
        nc = build_device_kernel(bacc.Bacc(None))
        if not nc.is_finalized():
            nc.finalize()  # Bacc.finalize = compile (wait legalization) + freeze
        _NC_CACHE["nc"] = nc
    return _NC_CACHE["nc"]


def _install_ntff_hook_shim():
    """Fabricate antenv.axon_hooks if the image lacks it (profiling only)."""
    import sys
    import types
    try:
        from antenv.axon_hooks import get_axon_ntff_profile_hook  # noqa: F401
        return
    except ImportError:
        pass
    from trn_agent_boot.trn_boot import _ntff_profile_via_ctypes
    hook = _ntff_profile_via_ctypes("/opt/axon/libaxon_pjrt.so")
    mod = types.ModuleType("antenv.axon_hooks")
    mod._hook = hook
    mod.get_axon_ntff_profile_hook = lambda: mod._hook
    mod.set_axon_ntff_profile_hook = lambda h: setattr(mod, "_hook", h)
    sys.modules["antenv.axon_hooks"] = mod
    import antenv
    antenv.axon_hooks = mod


def kernel(X, codewords, scale):
    from concourse.bass_utils import run_bass_kernel_spmd

    nc = _get_nc()
    in_maps = make_host_inputs(X, codewords, scale)
    trace = bool(int(os.environ.get("VQ_KERNEL_TRACE", "0")))
    kwargs = {}
    if trace:
        try:
            _install_ntff_hook_shim()
            tmpdir = os.environ.get("VQ_KERNEL_TMPDIR")
            if tmpdir:
                os.makedirs(tmpdir, exist_ok=True)
                kwargs["tmpdir"] = tmpdir
        except Exception as e:  # profiling must never break execution
            print(f"ntff hook install failed: {e}")
            trace = False
    res = run_bass_kernel_spmd(nc, in_maps, core_ids=list(range(NCORES)),
                               trace=trace, **kwargs)
    if trace and res.exec_time_ns is not None:
        print(f"HW exec time: {res.exec_time_ns} ns")
    return gather_output(res.results)


# revision 21
# speedup vs baseline: 1.4562x; 1.1278x over previous
"""Trainium2 Bass kernel for nn_EncodingP (vq_codebook soft-assignment encoding).

Reference computation (B=4, D=256, K=32, H=W=64, N=H*W=4096):
    Xf = X.reshape(B, D, N).transpose(0, 2, 1)            # (B, N, D)
    L[b,n,k] = ||x_bn||^2 - 2 <x_bn, c_k> + ||c_k||^2     # (B, N, K)
    A = softmax(L * scale, axis=-1)                        # (B, N, K)
    E[b,k,d] = sum_n A[b,n,k] * x_bn[d] - (sum_n A[b,n,k]) * c_k[d]

Sharding: 8 cores = 4 batches x 2 halves of N; host sums the two
half-partials per batch (E is linear in the n-sum).

Per-core dataflow:
  phase 1 (fp16 matmuls -> fp32 PSUM [128,512]; col-group j holds n-chunk j):
    psL[32j+k, nn] = -2*xc + x2     (x2 via an all-ones stationary over x^2)
  exp (fp32): expS = Exp(scale_k * psL + scale_k*c2_k)  (one ACT op with
    per-partition scale/bias; max |scale*L| ~ 79 < 88 so no max-subtract)
  transpose: 4 PE transposes of expS [128,128] slices -> layout A (araw)
  normalize per 128-col block: Z rowsum per 32-block, anorm = araw * (1/Z)
  phase 2 (4-way col-tiled): psE4[32g+k, :] += anorm_t^T @ xts_t  for the
    4 tiles t = 4g + c of group g (xts col 256 is ones -> Asum partials),
    then psE = stacked_I32^T @ psE4_evac (combine) + diag(-Asum) @ C
"""

import os

import numpy as np

import concourse.bass as bass
import concourse.tile as tile
from concourse import mybir
from concourse.masks import make_identity

B, D, K, H, W = 4, 256, 32, 64, 64
N = H * W            # 4096
NCORES = 8
NSH = B * N // NCORES  # 2048 positions per core
NT = NSH // 128        # 16 n-tiles per core
NAUG = D + 1           # 257: X^T columns + ones column

F32 = mybir.dt.float32
F16 = mybir.dt.float16

PH2_FP16 = bool(int(os.environ.get("VQ_PH2_FP16", "0")))

# cst16 (fp16) column layout
_CT0 = 0      # [0:32)    -2*C^T for d-block 0
_CT1 = 32     # [32:64)   -2*C^T for d-block 1
_ONE = 64     # [64:96)   ones
_C16 = 96     # [96:352)  codewords fp16 (partitions 0:32)
_S16 = 96 + D  # [352:384) stacked I32 (fp16)
_CF16 = _S16 + K
# cst32 (fp32) column layout
_SCL = 0
_BIA = 1
_C32 = 2           # [2:258)  codewords fp32 (partitions 0:32)
_S32 = 2 + D       # [258:290) stacked I32 (fp32): rows 32g+k, col k = 1
_CF32 = _S32 + K


def build_device_kernel(nc):
    ph2dt = F16 if PH2_FP16 else F32
    xdn_d = nc.declare_dram_parameter("xdn", [D, NSH], F16, isOutput=False)
    xta_d = nc.declare_dram_parameter("xta", [128, NT * NAUG], ph2dt,
                                      isOutput=False)
    c16_d = nc.declare_dram_parameter("cst16", [128, _CF16], F16, isOutput=False)
    c32_d = nc.declare_dram_parameter("cst32", [128, _CF32], F32, isOutput=False)
    out_d = nc.declare_dram_parameter("eout", [K, D], F32, isOutput=True)

    act = mybir.ActivationFunctionType
    alu = mybir.AluOpType

    with tile.TileContext(nc) as tc:
        with (
            tc.tile_pool(name="sb", bufs=1) as sb,
            tc.tile_pool(name="ps", bufs=1, space="PSUM") as ps,
            tc.tile_pool(name="psT", bufs=4, space="PSUM") as psT,
        ):
            cst16 = sb.tile([128, _CF16], F16)
            cst32 = sb.tile([128, _CF32], F32)
            x0 = sb.tile([128, NSH], F16)
            x1 = sb.tile([128, NSH], F16)
            sq0 = sb.tile([128, NSH], F16)
            sq1 = sb.tile([128, NSH], F16)
            xts = sb.tile([128, NT * NAUG], ph2dt)
            ident = sb.tile([128, 128], F32)

            # DMA issue: phase-1-critical tensors on the SP HWDGE ring,
            # the rest on the ACT ring.  One trigger per tensor chunk
            # (trigger dispatch costs ~0.6-0.8us each on the issuing queue).
            # all input DMAs ride the sync HWDGE ring, which drains FIFO:
            # issue order = bandwidth priority (x before xts)
            half = 8 * NAUG
            nc.sync.dma_start(out=cst16[:], in_=c16_d[:])
            nc.sync.dma_start(out=x0[:], in_=xdn_d[0:128, :])
            nc.sync.dma_start(out=x1[:], in_=xdn_d[128:256, :])
            nc.sync.dma_start(out=xts[:, 0:half], in_=xta_d[:, 0:half])
            nc.sync.dma_start(out=xts[:, half:2 * half],
                              in_=xta_d[:, half:2 * half])
            nc.scalar.dma_start(out=cst32[:], in_=c32_d[:])
            make_identity(nc, ident[:])

            # one-wait hygiene: absorb DMA/gpsimd completions into each
            # engine's program order early (several instruction types can
            # carry only one sync wait; extra waits cost EVSEM chains).
            dummy = ps.tile([1, 128], F32, tag="dummy")
            scr = sb.tile([128, 16], F32)
            nc.tensor.matmul(dummy[:, 0:16], cst16[:, 0:1], cst16[:, 0:16],
                             start=True, stop=True)
            # HAM warmup: ~4us of fp32 dummy matmuls on the identity while
            # the x16 DMA streams, so phase 1 runs at 2.4 GHz instead of 1.2
            for _ in range(6):
                nc.tensor.matmul(dummy[:], ident[:, 0:1], ident[:],
                                 start=True, stop=True)
            nc.scalar.copy(out=scr[:, 0:2], in_=cst32[:, 0:2])
            nc.scalar.copy(out=scr[:, 4:6], in_=ident[:, 0:2])
            nc.vector.tensor_copy(scr[:, 6:8], cst32[:, 0:2])

            # squares on device: sq = x^2 (fp16 out, fp32 internal); the two
            # chunks of each d-block go to different engines so a d-block's
            # squares finish in one op-latency.  The xts DMA triggers ride
            # the ACT queue between squares: they fire only once x0 has
            # landed, keeping early HBM bandwidth on the critical x stream.
            nc.scalar.square(out=sq0[:, 0:1024], in_=x0[:, 0:1024])
            nc.vector.tensor_mul(sq0[:, 1024:2048], x0[:, 1024:2048],
                                 x0[:, 1024:2048])
            nc.scalar.square(out=sq1[:, 0:1024], in_=x1[:, 0:1024])
            nc.vector.tensor_mul(sq1[:, 1024:2048], x1[:, 1024:2048],
                                 x1[:, 1024:2048])

            # phase 1: psL[32j+k, nn] = -2*xc + x2 for n = 512j + nn.
            # d-outer so all d0 matmuls can run before the d1 DMA lands;
            # interleaved starts across partition-disjoint col groups are
            # numerically fine (per-partition pending-zero), only the sim's
            # partition-blind group check needs skipping.
            psL = ps.tile([128, 512], F32, tag="psL")
            for d, xt_ in ((0, x0), (1, x1)):
                for j in range(4):
                    nc.tensor.matmul(
                        psL[32 * j:32 * (j + 1), :],
                        cst16[:, 32 * d:32 * (d + 1)],
                        xt_[:, 512 * j:512 * (j + 1)],
                        start=(d == 0), stop=False,
                        tile_position=(0, 32 * j), skip_group_check=True,
                    )
            for d, sq_ in ((0, sq0), (1, sq1)):
                for j in range(4):
                    nc.tensor.matmul(
                        psL[32 * j:32 * (j + 1), :],
                        cst16[:, _ONE:_ONE + 32],
                        sq_[:, 512 * j:512 * (j + 1)],
                        start=False, stop=(d == 1),
                        tile_position=(0, 32 * j), skip_group_check=True,
                    )

            expS = sb.tile([128, 512], F32)
            nc.scalar.activation(
                out=expS[:], in_=psL[:], func=act.Exp,
                bias=cst32[:, _BIA:_BIA + 1], scale=cst32[:, _SCL:_SCL + 1],
            )

            # transpose to layout A + per-block softmax normalization,
            # pipelined per 128-col block c
            araw = sb.tile([128, 512], F32)
            z = sb.tile([128, 16], F32)
            rz = sb.tile([128, 16], F32)
            anorm = sb.tile([128, 512], ph2dt)
            for c in range(4):
                pt = psT.tile([128, 128], F32, tag="pt")
                nc.tensor.transpose(pt[:], expS[:, 128 * c:128 * (c + 1)], ident[:])
                blk = slice(128 * c, 128 * (c + 1))
                if c % 2 == 0:
                    nc.scalar.copy(out=araw[:, blk], in_=pt[:])
                else:
                    nc.vector.tensor_copy(araw[:, blk], pt[:])
                zc = slice(4 * c, 4 * (c + 1))
                nc.vector.tensor_reduce(
                    z[:, zc], araw[:, blk].rearrange("p (g k) -> p g k", k=K),
                    axis=mybir.AxisListType.X, op=alu.add,
                )
                nc.vector.reciprocal(rz[:, zc], z[:, zc])
                nc.vector.tensor_tensor(
                    out=anorm[:, blk].rearrange("p (g k) -> p g k", k=K),
                    in0=araw[:, blk].rearrange("p (g k) -> p g k", k=K),
                    in1=rz[:, zc].rearrange("p (g x) -> p g x", x=1).broadcast_to(
                        [128, 4, K]),
                    op=alu.mult,
                )

            # keep the PE HAM busy across the normalize window
            for _ in range(3):
                nc.tensor.matmul(dummy[:], ident[:, 0:1], ident[:],
                                 start=True, stop=True)
            # absorb the xts DMA completions into PE program order
            nc.tensor.matmul(dummy[:, 0:16], xts[:, 0:1], xts[:, 0:16],
                             start=True, stop=True)
            nc.tensor.matmul(dummy[:, 0:16], xts[:, half:half + 1],
                             xts[:, half:half + 16], start=True, stop=True)
            if not PH2_FP16:
                nc.tensor.matmul(dummy[:, 0:16], cst32[:, _C32:_C32 + 1],
                                 cst32[:, _C32:_C32 + 16],
                                 start=True, stop=True)

            # phase 2, 4-way col-tiled: group g accumulates tiles t = 4g + c
            # into psE4[32g:32g+32, :]; c-major order so the 4 groups run
            # concurrently in disjoint 32-col array strips
            # free width 272 (not 257): 32-partition slice offsets must be
            # 2048-byte aligned for PSUM pending-zero bookkeeping
            psE4 = ps.tile([128, 272], F32, tag="psE4")
            for c in range(4):
                for g in range(4):
                    t = 4 * g + c
                    col = 128 * c + 32 * g
                    nc.tensor.matmul(
                        psE4[32 * g:32 * (g + 1), 0:NAUG],
                        anorm[:, col:col + 32],
                        xts[:, NAUG * t:NAUG * (t + 1)],
                        start=(c == 0), stop=(c == 3),
                        tile_position=(0, 32 * g), skip_group_check=True,
                    )

            # combine the 4 partial groups: psE = stacked_I32^T @ psE4
            full4 = sb.tile([128, NAUG], F32)
            nc.scalar.copy(out=full4[:], in_=psE4[:, 0:NAUG])
            psE = ps.tile([K, NAUG], F32, tag="psE")
            stk = cst16[:, _S16:_S16 + K] if PH2_FP16 \
                else cst32[:, _S32:_S32 + K]
            stk_rhs = full4[:]
            if PH2_FP16:
                # matmul operands must share dtype; keep combine in fp32
                stk = cst32[:, _S32:_S32 + K]
            nc.tensor.matmul(psE[:], stk, stk_rhs, start=True, stop=True)

            # E = psE[:, 0:256] - Asum * C via one more accumulating matmul
            asum = sb.tile([K, 1], F32)
            nc.vector.tensor_scalar_mul(asum[:], psE[:, D:D + 1], -1.0)
            diagna = sb.tile([K, K], F32)
            nc.vector.tensor_scalar_mul(diagna[:], ident[0:K, 0:K], asum[:])
            nc.tensor.matmul(psE[:, 0:D], diagna[:], cst32[0:K, _C32:_C32 + D],
                             start=False, stop=False, skip_group_check=True)
            esb = sb.tile([K, D], F32)
            nc.scalar.copy(out=esb[:], in_=psE[:, 0:D])
            nc.scalar.dma_start(out=out_d[:], in_=esb[:])

    return nc


def make_host_inputs(X, codewords, scale):
    """Shard + lay out inputs for the 8 cores. Returns list of in_maps."""
    X = np.ascontiguousarray(X, dtype=np.float32)
    codewords = np.asarray(codewords, dtype=np.float32)
    scale = np.asarray(scale, dtype=np.float32)
    ph2np = np.float16 if PH2_FP16 else np.float32

    stacked = np.zeros((128, K), dtype=np.float32)
    for g in range(4):
        stacked[32 * g:32 * (g + 1)] = np.eye(K, dtype=np.float32)

    c2 = (codewords.astype(np.float64) ** 2).sum(axis=1)
    cst16 = np.zeros((128, _CF16), dtype=np.float16)
    ctn2 = (-2.0 * codewords.T).astype(np.float16)        # [D, K]
    cst16[:, _CT0:_CT0 + K] = ctn2[0:128]
    cst16[:, _CT1:_CT1 + K] = ctn2[128:256]
    cst16[:, _ONE:_ONE + K] = 1.0
    cst16[0:K, _C16:_C16 + D] = codewords.astype(np.float16)
    cst16[:, _S16:_S16 + K] = stacked.astype(np.float16)
    cst32 = np.zeros((128, _CF32), dtype=np.float32)
    cst32[:, _SCL] = np.tile(scale, 4)
    cst32[:, _BIA] = np.tile((scale.astype(np.float64) * c2).astype(np.float32), 4)
    cst32[0:K, _C32:_C32 + D] = codewords
    cst32[:, _S32:_S32 + K] = stacked

    Xr = X.reshape(B, D, N)
    in_maps = []
    for core in range(NCORES):
        b, h = core // 2, core % 2
        xdn = np.ascontiguousarray(Xr[b][:, NSH * h:NSH * (h + 1)])
        xdn16 = xdn.astype(np.float16)
        xt = np.ascontiguousarray(xdn.T)                  # [NSH, D] fp32
        xta = np.concatenate(
            [xt, np.ones((NSH, 1), dtype=np.float32)], axis=1).astype(ph2np)
        xta_dev = np.ascontiguousarray(
            xta.reshape(NT, 128, NAUG).transpose(1, 0, 2).reshape(128, NT * NAUG))
        in_maps.append({"xdn": xdn16, "xta": xta_dev,
                        "cst16": cst16, "cst32": cst32})
    return in_maps


def gather_output(results):
    E = np.empty((B, K, D), dtype=np.float32)
    for b in range(B):
        E[b] = results[2 * b]["eout"] + results[2 * b + 1]["eout"]
    return E


_NC_CACHE = {}


def _get_nc():
    if "nc" not in _NC_CACHE:
        # Bacc (not plain Bass): its compile() runs the TRN2 sync-wait
        # legalization (max 1 wait per instruction) that walrus requires.
        from concourse import bacc
        nc = build_device_kernel(bacc.Bacc(None))
        if not nc.is_finalized():
            nc.finalize()  # Bacc.finalize = compile (wait legalization) + freeze
        _NC_CACHE["nc"] = nc
    return _NC_CACHE["nc"]


def _install_ntff_hook_shim():
    """Fabricate antenv.axon_hooks if the image lacks it (profiling only)."""
    import sys
    import types
    try:
        from antenv.axon_hooks import get_axon_ntff_profile_hook  # noqa: F401
        return
    except ImportError:
        pass
    from trn_agent_boot.trn_boot import _ntff_profile_via_ctypes
    hook = _ntff_profile_via_ctypes("/opt/axon/libaxon_pjrt.so")
    mod = types.ModuleType("antenv.axon_hooks")
    mod._hook = hook
    mod.get_axon_ntff_profile_hook = lambda: mod._hook
    mod.set_axon_ntff_profile_hook = lambda h: setattr(mod, "_hook", h)
    sys.modules["antenv.axon_hooks"] = mod
    import antenv
    antenv.axon_hooks = mod


def kernel(X, codewords, scale):
    from concourse.bass_utils import run_bass_kernel_spmd

    nc = _get_nc()
    in_maps = make_host_inputs(X, codewords, scale)
    trace = bool(int(os.environ.get("VQ_KERNEL_TRACE", "0")))
    kwargs = {}
    if trace:
        try:
            _install_ntff_hook_shim()
            tmpdir = os.environ.get("VQ_KERNEL_TMPDIR")
            if tmpdir:
                os.makedirs(tmpdir, exist_ok=True)
                kwargs["tmpdir"] = tmpdir
        except Exception as e:  # profiling must never break execution
            print(f"ntff hook install failed: {e}")
            trace = False
    res = run_bass_kernel_spmd(nc, in_maps, core_ids=list(range(NCORES)),
                               trace=trace, **kwargs)
    if trace and res.exec_time_ns is not None:
        print(f"HW exec time: {res.exec_time_ns} ns")
    return gather_output(res.results)


# revision 22
# speedup vs baseline: 1.5015x; 1.0311x over previous
"""Trainium2 Bass kernel for nn_EncodingP (vq_codebook soft-assignment encoding).

Reference computation (B=4, D=256, K=32, H=W=64, N=H*W=4096):
    Xf = X.reshape(B, D, N).transpose(0, 2, 1)            # (B, N, D)
    L[b,n,k] = ||x_bn||^2 - 2 <x_bn, c_k> + ||c_k||^2     # (B, N, K)
    A = softmax(L * scale, axis=-1)                        # (B, N, K)
    E[b,k,d] = sum_n A[b,n,k] * x_bn[d] - (sum_n A[b,n,k]) * c_k[d]

Sharding: 8 cores = 4 batches x 2 halves of N; host sums the two
half-partials per batch (E is linear in the n-sum).

Per-core dataflow:
  phase 1 (fp16 matmuls -> fp32 PSUM [128,512]; col-group j holds n-chunk j):
    psL[32j+k, nn] = -2*xc + x2     (x2 via an all-ones stationary over x^2)
  exp (fp32): expS = Exp(scale_k * psL + scale_k*c2_k)  (one ACT op with
    per-partition scale/bias; max |scale*L| ~ 79 < 88 so no max-subtract)
  transpose: 4 PE transposes of expS [128,128] slices -> layout A (araw)
  normalize per 128-col block: Z rowsum per 32-block, anorm = araw * (1/Z)
  phase 2 (4-way col-tiled): psE4[32g+k, :] += anorm_t^T @ xts_t  for the
    4 tiles t = 4g + c of group g (xts col 256 is ones -> Asum partials),
    then psE = stacked_I32^T @ psE4_evac (combine) + diag(-Asum) @ C
"""

import os

import numpy as np

import concourse.bass as bass
import concourse.tile as tile
from concourse import mybir
from concourse.masks import make_identity

B, D, K, H, W = 4, 256, 32, 64, 64
N = H * W            # 4096
NCORES = 8
NSH = B * N // NCORES  # 2048 positions per core
NT = NSH // 128        # 16 n-tiles per core
NAUG = D + 1           # 257: X^T columns + ones column

F32 = mybir.dt.float32
F16 = mybir.dt.float16

PH2_FP16 = bool(int(os.environ.get("VQ_PH2_FP16", "0")))

# cst16 (fp16) column layout
_CT0 = 0      # [0:32)    -2*C^T for d-block 0
_CT1 = 32     # [32:64)   -2*C^T for d-block 1
_ONE = 64     # [64:96)   ones
_CF16 = 96
# cst32 (fp32) column layout
_SCL = 0
_BIA = 1
_CF32 = 2


def build_device_kernel(nc):
    ph2dt = F16 if PH2_FP16 else F32
    xdn_d = nc.declare_dram_parameter("xdn", [D, NSH], F16, isOutput=False)
    xta_d = nc.declare_dram_parameter("xta", [128, NT * NAUG], ph2dt,
                                      isOutput=False)
    c16_d = nc.declare_dram_parameter("cst16", [128, _CF16], F16, isOutput=False)
    c32_d = nc.declare_dram_parameter("cst32", [128, _CF32], F32, isOutput=False)
    out_d = nc.declare_dram_parameter("eout", [128, NAUG], F32, isOutput=True)

    act = mybir.ActivationFunctionType
    alu = mybir.AluOpType

    with tile.TileContext(nc) as tc:
        with (
            tc.tile_pool(name="sb", bufs=1) as sb,
            tc.tile_pool(name="ps", bufs=1, space="PSUM") as ps,
            tc.tile_pool(name="psT", bufs=4, space="PSUM") as psT,
        ):
            cst16 = sb.tile([128, _CF16], F16)
            cst32 = sb.tile([128, _CF32], F32)
            x0 = sb.tile([128, NSH], F16)
            x1 = sb.tile([128, NSH], F16)
            sq0a = sb.tile([128, 1024], F16)
            sq0b = sb.tile([128, 1024], F16)
            sq1a = sb.tile([128, 1024], F16)
            sq1b = sb.tile([128, 1024], F16)
            xts = sb.tile([128, NT * NAUG], ph2dt)
            ident = sb.tile([128, 128], F32)

            # DMA issue: phase-1-critical tensors on the SP HWDGE ring,
            # the rest on the ACT ring.  One trigger per tensor chunk
            # (trigger dispatch costs ~0.6-0.8us each on the issuing queue).
            # all input DMAs ride the sync HWDGE ring, which drains FIFO:
            # issue order = bandwidth priority (x before xts)
            half = 8 * NAUG
            nc.sync.dma_start(out=cst16[:], in_=c16_d[:])
            nc.sync.dma_start(out=x0[:], in_=xdn_d[0:128, :])
            nc.sync.dma_start(out=x1[:], in_=xdn_d[128:256, :])
            nc.sync.dma_start(out=xts[:, 0:half], in_=xta_d[:, 0:half])
            nc.sync.dma_start(out=xts[:, half:2 * half],
                              in_=xta_d[:, half:2 * half])
            nc.scalar.dma_start(out=cst32[:], in_=c32_d[:])
            make_identity(nc, ident[:])

            # one-wait hygiene: absorb DMA/gpsimd completions into each
            # engine's program order early (several instruction types can
            # carry only one sync wait; extra waits cost EVSEM chains).
            dummy = ps.tile([1, 128], F32, tag="dummy")
            scr = sb.tile([128, 16], F32)
            nc.tensor.matmul(dummy[:, 0:16], cst16[:, 0:1], cst16[:, 0:16],
                             start=True, stop=True)
            # HAM warmup: ~4us of fp32 dummy matmuls on the identity while
            # the x16 DMA streams, so phase 1 runs at 2.4 GHz instead of 1.2
            for _ in range(6):
                nc.tensor.matmul(dummy[:], ident[:, 0:1], ident[:],
                                 start=True, stop=True)
            nc.scalar.copy(out=scr[:, 0:2], in_=cst32[:, 0:2])
            nc.scalar.copy(out=scr[:, 4:6], in_=ident[:, 0:2])
            nc.vector.tensor_copy(scr[:, 6:8], cst32[:, 0:2])

            # squares on device: sq = x^2 (fp16 out, fp32 internal); the two
            # chunks of each d-block go to different engines so a d-block's
            # squares finish in one op-latency.  The xts DMA triggers ride
            # the ACT queue between squares: they fire only once x0 has
            # landed, keeping early HBM bandwidth on the critical x stream.
            nc.scalar.square(out=sq0a[:], in_=x0[:, 0:1024])
            nc.vector.tensor_mul(sq0b[:], x0[:, 1024:2048], x0[:, 1024:2048])
            nc.scalar.square(out=sq1a[:], in_=x1[:, 0:1024])
            nc.vector.tensor_mul(sq1b[:], x1[:, 1024:2048], x1[:, 1024:2048])

            # phase 1: psL[32j+k, nn] = -2*xc + x2 for n = 512j + nn.
            # d-outer so all d0 matmuls can run before the d1 DMA lands;
            # interleaved starts across partition-disjoint col groups are
            # numerically fine (per-partition pending-zero), only the sim's
            # partition-blind group check needs skipping.
            psL = ps.tile([128, 512], F32, tag="psL")
            for d, xt_ in ((0, x0), (1, x1)):
                for j in range(4):
                    nc.tensor.matmul(
                        psL[32 * j:32 * (j + 1), :],
                        cst16[:, 32 * d:32 * (d + 1)],
                        xt_[:, 512 * j:512 * (j + 1)],
                        start=(d == 0), stop=False,
                        tile_position=(0, 32 * j), skip_group_check=True,
                    )
            sqmap = {0: (sq0a, sq0b), 1: (sq1a, sq1b)}
            for d in range(2):
                for j in range(4):
                    sq_ = sqmap[d][j // 2]
                    nc.tensor.matmul(
                        psL[32 * j:32 * (j + 1), :],
                        cst16[:, _ONE:_ONE + 32],
                        sq_[:, 512 * (j % 2):512 * (j % 2 + 1)],
                        start=False, stop=(d == 1),
                        tile_position=(0, 32 * j), skip_group_check=True,
                    )

            expS = sb.tile([128, 512], F32)
            nc.scalar.activation(
                out=expS[:], in_=psL[:], func=act.Exp,
                bias=cst32[:, _BIA:_BIA + 1], scale=cst32[:, _SCL:_SCL + 1],
            )

            # transpose to layout A + per-block softmax normalization,
            # pipelined per 128-col block c
            araw = sb.tile([128, 512], F32)
            z = sb.tile([128, 16], F32)
            rz = sb.tile([128, 16], F32)
            anorm = sb.tile([128, 512], ph2dt)
            for c in range(4):
                pt = psT.tile([128, 128], F32, tag="pt")
                nc.tensor.transpose(pt[:], expS[:, 128 * c:128 * (c + 1)], ident[:])
                blk = slice(128 * c, 128 * (c + 1))
                if c % 2 == 0:
                    nc.scalar.copy(out=araw[:, blk], in_=pt[:])
                else:
                    nc.vector.tensor_copy(araw[:, blk], pt[:])
                zc = slice(4 * c, 4 * (c + 1))
                nc.vector.tensor_reduce(
                    z[:, zc], araw[:, blk].rearrange("p (g k) -> p g k", k=K),
                    axis=mybir.AxisListType.X, op=alu.add,
                )
                nc.vector.reciprocal(rz[:, zc], z[:, zc])
                nc.vector.tensor_tensor(
                    out=anorm[:, blk].rearrange("p (g k) -> p g k", k=K),
                    in0=araw[:, blk].rearrange("p (g k) -> p g k", k=K),
                    in1=rz[:, zc].rearrange("p (g x) -> p g x", x=1).broadcast_to(
                        [128, 4, K]),
                    op=alu.mult,
                )

            # keep the PE HAM busy across the normalize window (anchored on
            # araw so the scheduler cannot hoist them earlier)
            for c in range(3):
                nc.tensor.matmul(dummy[:], araw[:, 128 * c:128 * c + 1],
                                 araw[:, 128 * c:128 * (c + 1)],
                                 start=True, stop=True)
            # absorb the xts DMA completions into PE program order
            nc.tensor.matmul(dummy[:, 0:16], xts[:, 0:1], xts[:, 0:16],
                             start=True, stop=True)
            nc.tensor.matmul(dummy[:, 0:16], xts[:, half:half + 1],
                             xts[:, half:half + 16], start=True, stop=True)

            # phase 2, 4-way col-tiled: group g accumulates tiles t = 4g + c
            # into psE4[32g:32g+32, :]; c-major order so the 4 groups run
            # concurrently in disjoint 32-col array strips
            # free width 272 (not 257): 32-partition slice offsets must be
            # 2048-byte aligned for PSUM pending-zero bookkeeping
            psE4 = ps.tile([128, 272], F32, tag="psE4")
            for c in range(4):
                for g in range(4):
                    t = 4 * g + c
                    col = 128 * c + 32 * g
                    nc.tensor.matmul(
                        psE4[32 * g:32 * (g + 1), 0:NAUG],
                        anorm[:, col:col + 32],
                        xts[:, NAUG * t:NAUG * (t + 1)],
                        start=(c == 0), stop=(c == 3),
                        tile_position=(0, 32 * g), skip_group_check=True,
                    )

            # evacuate the 4-group partials; the host does the final
            # 4-way sum and the -Asum*C correction (tiny)
            full4 = sb.tile([128, NAUG], F32)
            nc.scalar.copy(out=full4[:], in_=psE4[:, 0:NAUG])
            nc.scalar.dma_start(out=out_d[:], in_=full4[:])

    return nc


def make_host_inputs(X, codewords, scale):
    """Shard + lay out inputs for the 8 cores. Returns list of in_maps."""
    X = np.ascontiguousarray(X, dtype=np.float32)
    codewords = np.asarray(codewords, dtype=np.float32)
    scale = np.asarray(scale, dtype=np.float32)
    ph2np = np.float16 if PH2_FP16 else np.float32

    c2 = (codewords.astype(np.float64) ** 2).sum(axis=1)
    cst16 = np.zeros((128, _CF16), dtype=np.float16)
    ctn2 = (-2.0 * codewords.T).astype(np.float16)        # [D, K]
    cst16[:, _CT0:_CT0 + K] = ctn2[0:128]
    cst16[:, _CT1:_CT1 + K] = ctn2[128:256]
    cst16[:, _ONE:_ONE + K] = 1.0
    cst32 = np.zeros((128, _CF32), dtype=np.float32)
    cst32[:, _SCL] = np.tile(scale, 4)
    cst32[:, _BIA] = np.tile((scale.astype(np.float64) * c2).astype(np.float32), 4)

    Xr = X.reshape(B, D, N)
    in_maps = []
    for core in range(NCORES):
        b, h = core // 2, core % 2
        xdn = np.ascontiguousarray(Xr[b][:, NSH * h:NSH * (h + 1)])
        xdn16 = xdn.astype(np.float16)
        xt = np.ascontiguousarray(xdn.T)                  # [NSH, D] fp32
        xta = np.concatenate(
            [xt, np.ones((NSH, 1), dtype=np.float32)], axis=1).astype(ph2np)
        xta_dev = np.ascontiguousarray(
            xta.reshape(NT, 128, NAUG).transpose(1, 0, 2).reshape(128, NT * NAUG))
        in_maps.append({"xdn": xdn16, "xta": xta_dev,
                        "cst16": cst16, "cst32": cst32})
    return in_maps


def gather_output(results, codewords):
    E = np.zeros((B, K, D), dtype=np.float32)
    for core, res in enumerate(results):
        full4 = res["eout"].reshape(4, K, NAUG)
        part = full4.sum(axis=0)                      # [K, NAUG]
        E[core // 2] += part[:, 0:D] - part[:, D:D + 1] * codewords
    return E


_NC_CACHE = {}


def _get_nc():
    if "nc" not in _NC_CACHE:
        # Bacc (not plain Bass): its compile() runs the TRN2 sync-wait
        # legalization (max 1 wait per instruction) that walrus requires.
        from concourse import bacc
        nc = build_device_kernel(bacc.Bacc(None))
        if not nc.is_finalized():
            nc.finalize()  # Bacc.finalize = compile (wait legalization) + freeze
        _NC_CACHE["nc"] = nc
    return _NC_CACHE["nc"]


def _install_ntff_hook_shim():
    """Fabricate antenv.axon_hooks if the image lacks it (profiling only)."""
    import sys
    import types
    try:
        from antenv.axon_hooks import get_axon_ntff_profile_hook  # noqa: F401
        return
    except ImportError:
        pass
    from trn_agent_boot.trn_boot import _ntff_profile_via_ctypes
    hook = _ntff_profile_via_ctypes("/opt/axon/libaxon_pjrt.so")
    mod = types.ModuleType("antenv.axon_hooks")
    mod._hook = hook
    mod.get_axon_ntff_profile_hook = lambda: mod._hook
    mod.set_axon_ntff_profile_hook = lambda h: setattr(mod, "_hook", h)
    sys.modules["antenv.axon_hooks"] = mod
    import antenv
    antenv.axon_hooks = mod


def kernel(X, codewords, scale):
    from concourse.bass_utils import run_bass_kernel_spmd

    nc = _get_nc()
    in_maps = make_host_inputs(X, codewords, scale)
    trace = bool(int(os.environ.get("VQ_KERNEL_TRACE", "0")))
    kwargs = {}
    if trace:
        try:
            _install_ntff_hook_shim()
            tmpdir = os.environ.get("VQ_KERNEL_TMPDIR")
            if tmpdir:
                os.makedirs(tmpdir, exist_ok=True)
                kwargs["tmpdir"] = tmpdir
        except Exception as e:  # profiling must never break execution
            print(f"ntff hook install failed: {e}")
            trace = False
    res = run_bass_kernel_spmd(nc, in_maps, core_ids=list(range(NCORES)),
                               trace=trace, **kwargs)
    if trace and res.exec_time_ns is not None:
        print(f"HW exec time: {res.exec_time_ns} ns")
    return gather_output(res.results, np.asarray(codewords, np.float32))


# revision 28
# speedup vs baseline: 1.5877x; 1.0574x over previous
"""Trainium2 Bass kernel for nn_EncodingP (vq_codebook soft-assignment encoding).

Reference computation (B=4, D=256, K=32, H=W=64, N=H*W=4096):
    Xf = X.reshape(B, D, N).transpose(0, 2, 1)            # (B, N, D)
    L[b,n,k] = ||x_bn||^2 - 2 <x_bn, c_k> + ||c_k||^2     # (B, N, K)
    A = softmax(L * scale, axis=-1)                        # (B, N, K)
    E[b,k,d] = sum_n A[b,n,k] * x_bn[d] - (sum_n A[b,n,k]) * c_k[d]

Sharding: 8 cores = 4 batches x 2 halves of N; host sums the two
half-partials per batch (E is linear in the n-sum).

Per-core dataflow:
  phase 1 (fp16 matmuls -> fp32 PSUM [128,512]; col-group j holds n-chunk j):
    psL[32j+k, nn] = -2*xc + x2     (x2 via an all-ones stationary over x^2)
  exp (fp32): expS = Exp(scale_k * psL + scale_k*c2_k)  (one ACT op with
    per-partition scale/bias; max |scale*L| ~ 79 < 88 so no max-subtract)
  transpose: 4 PE transposes of expS [128,128] slices -> layout A (araw)
  normalize per 128-col block: Z rowsum per 32-block, anorm = araw * (1/Z)
  phase 2 (4-way col-tiled): psE4[32g+k, :] += anorm_t^T @ xts_t  for the
    4 tiles t = 4g + c of group g (xts col 256 is ones -> Asum partials),
    then psE = stacked_I32^T @ psE4_evac (combine) + diag(-Asum) @ C
"""

import os

import numpy as np

import concourse.bass as bass
import concourse.tile as tile
from concourse import mybir
from concourse.masks import make_identity

B, D, K, H, W = 4, 256, 32, 64, 64
N = H * W            # 4096
NCORES = 8
NSH = B * N // NCORES  # 2048 positions per core
NT = NSH // 128        # 16 n-tiles per core
NAUG = D + 1           # 257: X^T columns + ones column

F32 = mybir.dt.float32
F16 = mybir.dt.float16

PH2_FP16 = bool(int(os.environ.get("VQ_PH2_FP16", "0")))

# cst16 (fp16) column layout
_CT0 = 0      # [0:32)    -2*C^T for d-block 0
_CT1 = 32     # [32:64)   -2*C^T for d-block 1
_ONE = 64     # [64:96)   ones
_CF16 = 96
# cst32 (fp32) column layout
_SCL = 0
_BIA = 1
_CF32 = 2


def build_device_kernel(nc):
    ph2dt = F16 if PH2_FP16 else F32
    xdn_d = nc.declare_dram_parameter("xdn", [D, NSH], F16, isOutput=False)
    xta_d = nc.declare_dram_parameter("xta", [128, NT * NAUG], ph2dt,
                                      isOutput=False)
    c16_d = nc.declare_dram_parameter("cst16", [128, _CF16], F16, isOutput=False)
    c32_d = nc.declare_dram_parameter("cst32", [128, _CF32], F32, isOutput=False)
    out_d = nc.declare_dram_parameter("eout", [128, NAUG], F32, isOutput=True)

    act = mybir.ActivationFunctionType
    alu = mybir.AluOpType

    with tile.TileContext(nc) as tc:
        with (
            tc.tile_pool(name="sb", bufs=1) as sb,
            tc.tile_pool(name="ps", bufs=1, space="PSUM") as ps,
            tc.tile_pool(name="psT", bufs=4, space="PSUM") as psT,
        ):
            cst16 = sb.tile([128, _CF16], F16)
            cst32 = sb.tile([128, _CF32], F32)
            x0 = sb.tile([128, NSH], F16)
            x1 = sb.tile([128, NSH], F16)
            sq0a = sb.tile([128, 1024], F16)
            sq0b = sb.tile([128, 1024], F16)
            sq1a = sb.tile([128, 1024], F16)
            sq1b = sb.tile([128, 1024], F16)
            xts = sb.tile([128, NT * NAUG], ph2dt)
            ident = sb.tile([128, 128], F32)

            # all input DMAs ride the sync HWDGE ring, which drains FIFO:
            # issue order = bandwidth priority (x before xts)
            half = 8 * NAUG
            nc.sync.dma_start(out=cst16[:], in_=c16_d[:])
            nc.sync.dma_start(out=x0[:], in_=xdn_d[0:128, :])
            nc.sync.dma_start(out=x1[:], in_=xdn_d[128:256, :])
            nc.sync.dma_start(out=xts[:, 0:half], in_=xta_d[:, 0:half])
            nc.sync.dma_start(out=xts[:, half:2 * half],
                              in_=xta_d[:, half:2 * half])
            nc.scalar.dma_start(out=cst32[:], in_=c32_d[:])
            make_identity(nc, ident[:])

            # one-wait hygiene: absorb DMA/gpsimd completions into each
            # engine's program order early (several instruction types can
            # carry only one sync wait; extra waits cost EVSEM chains).
            dummy = ps.tile([1, 128], F32, tag="dummy")
            scr = sb.tile([128, 16], F32)
            nc.tensor.matmul(dummy[:, 0:16], cst16[:, 0:1], cst16[:, 0:16],
                             start=True, stop=True)
            # HAM warmup: ~3us of fp32 dummy matmuls on the identity while
            # the x DMA streams, so phase 1 runs at 2.4 GHz instead of 1.2
            for _ in range(6):
                nc.tensor.matmul(dummy[:], ident[:, 0:1], ident[:],
                                 start=True, stop=True)
            nc.scalar.copy(out=scr[:, 0:2], in_=cst32[:, 0:2])
            nc.scalar.copy(out=scr[:, 4:6], in_=ident[:, 0:2])
            nc.vector.tensor_copy(scr[:, 6:8], cst32[:, 0:2])

            # squares on device: sq = x^2 (fp16 out, fp32 internal); the two
            # chunks of each d-block go to different engines so a d-block's
            # squares finish in one op-latency
            nc.scalar.square(out=sq0a[:], in_=x0[:, 0:1024])
            nc.vector.tensor_mul(sq0b[:], x0[:, 1024:2048], x0[:, 1024:2048])
            nc.scalar.square(out=sq1a[:], in_=x1[:, 0:1024])
            nc.vector.tensor_mul(sq1b[:], x1[:, 1024:2048], x1[:, 1024:2048])

            # phase 1: psL[32j+k, nn] = -2*xc + x2 for n = 512j + nn.
            # d-outer so all d0 matmuls can run before the d1 DMA lands;
            # interleaved starts across partition-disjoint col groups are
            # numerically fine (per-partition pending-zero), only the sim's
            # partition-blind group check needs skipping.
            psL = ps.tile([128, 512], F32, tag="psL")
            for d, xt_ in ((0, x0), (1, x1)):
                for j in range(4):
                    nc.tensor.matmul(
                        psL[32 * j:32 * (j + 1), :],
                        cst16[:, 32 * d:32 * (d + 1)],
                        xt_[:, 512 * j:512 * (j + 1)],
                        start=(d == 0), stop=False,
                        tile_position=(0, 32 * j), skip_group_check=True,
                    )
            sqmap = {0: (sq0a, sq0b), 1: (sq1a, sq1b)}
            for d in range(2):
                for j in range(4):
                    sq_ = sqmap[d][j // 2]
                    nc.tensor.matmul(
                        psL[32 * j:32 * (j + 1), :],
                        cst16[:, _ONE:_ONE + 32],
                        sq_[:, 512 * (j % 2):512 * (j % 2 + 1)],
                        start=False, stop=(d == 1),
                        tile_position=(0, 32 * j), skip_group_check=True,
                    )

            expS = sb.tile([128, 512], F32)

            # transpose to layout A + per-block softmax normalization,
            # pipelined per 128-col block c (exp also split per block so the
            # first transpose starts one small-op-latency after phase 1)
            araw = sb.tile([128, 512], F32)
            z = sb.tile([128, 16], F32)
            rz = sb.tile([128, 16], F32)
            anorm = sb.tile([128, 512], ph2dt)
            for c in range(4):
                nc.scalar.activation(
                    out=expS[:, 128 * c:128 * (c + 1)],
                    in_=psL[:, 128 * c:128 * (c + 1)], func=act.Exp,
                    bias=cst32[:, _BIA:_BIA + 1], scale=cst32[:, _SCL:_SCL + 1],
                )
                pt = psT.tile([128, 128], F32, tag="pt")
                nc.tensor.transpose(pt[:], expS[:, 128 * c:128 * (c + 1)], ident[:])
                blk = slice(128 * c, 128 * (c + 1))
                if c % 2 == 0:
                    nc.scalar.copy(out=araw[:, blk], in_=pt[:])
                else:
                    nc.vector.tensor_copy(araw[:, blk], pt[:])
                zc = slice(4 * c, 4 * (c + 1))
                nc.vector.tensor_reduce(
                    z[:, zc], araw[:, blk].rearrange("p (g k) -> p g k", k=K),
                    axis=mybir.AxisListType.X, op=alu.add,
                )
                nc.vector.reciprocal(rz[:, zc], z[:, zc])
                nc.vector.tensor_tensor(
                    out=anorm[:, blk].rearrange("p (g k) -> p g k", k=K),
                    in0=araw[:, blk].rearrange("p (g k) -> p g k", k=K),
                    in1=rz[:, zc].rearrange("p (g x) -> p g x", x=1).broadcast_to(
                        [128, 4, K]),
                    op=alu.mult,
                )

            # keep the PE HAM busy across the normalize window (anchored on
            # araw so the scheduler cannot hoist them earlier)
            for c in range(3):
                nc.tensor.matmul(dummy[:], araw[:, 128 * c:128 * c + 1],
                                 araw[:, 128 * c:128 * (c + 1)],
                                 start=True, stop=True)
            # absorb the xts DMA completions into PE program order
            nc.tensor.matmul(dummy[:, 0:16], xts[:, 0:1], xts[:, 0:16],
                             start=True, stop=True)
            nc.tensor.matmul(dummy[:, 0:16], xts[:, half:half + 1],
                             xts[:, half:half + 16], start=True, stop=True)

            # phase 2, 4-way col-tiled: group g accumulates tiles t = 4g + c
            # into psE4[32g:32g+32, :]; c-major order so the 4 groups run
            # concurrently in disjoint 32-col array strips.
            # free width 272 (not 257): 32-partition slice offsets must be
            # 2048-byte aligned for PSUM pending-zero bookkeeping
            psE4 = ps.tile([128, 272], F32)
            for c in range(4):
                for g in range(4):
                    t = 4 * g + c
                    col = 128 * c + 32 * g
                    nc.tensor.matmul(
                        psE4[32 * g:32 * (g + 1), 0:NAUG],
                        anorm[:, col:col + 32],
                        xts[:, NAUG * t:NAUG * (t + 1)],
                        start=(c == 0), stop=(c == 3),
                        tile_position=(0, 32 * g), skip_group_check=True,
                    )

            # evacuate the 4-group partials; the host does the final
            # 4-way sum and the -Asum*C correction (tiny)
            full4 = sb.tile([128, NAUG], F32)
            nc.scalar.copy(out=full4[:], in_=psE4[:, 0:NAUG])
            nc.scalar.dma_start(out=out_d[:], in_=full4[:])

    return nc


def make_host_inputs(X, codewords, scale):
    """Shard + lay out inputs for the 8 cores. Returns list of in_maps."""
    X = np.ascontiguousarray(X, dtype=np.float32)
    codewords = np.asarray(codewords, dtype=np.float32)
    scale = np.asarray(scale, dtype=np.float32)
    ph2np = np.float16 if PH2_FP16 else np.float32

    c2 = (codewords.astype(np.float64) ** 2).sum(axis=1)
    cst16 = np.zeros((128, _CF16), dtype=np.float16)
    ctn2 = (-2.0 * codewords.T).astype(np.float16)        # [D, K]
    cst16[:, _CT0:_CT0 + K] = ctn2[0:128]
    cst16[:, _CT1:_CT1 + K] = ctn2[128:256]
    cst16[:, _ONE:_ONE + K] = 1.0
    cst32 = np.zeros((128, _CF32), dtype=np.float32)
    cst32[:, _SCL] = np.tile(scale, 4)
    cst32[:, _BIA] = np.tile((scale.astype(np.float64) * c2).astype(np.float32), 4)

    Xr = X.reshape(B, D, N)
    in_maps = []
    for core in range(NCORES):
        b, h = core // 2, core % 2
        xdn = np.ascontiguousarray(Xr[b][:, NSH * h:NSH * (h + 1)])
        xdn16 = xdn.astype(np.float16)
        xt = np.ascontiguousarray(xdn.T)                  # [NSH, D] fp32
        xta = np.concatenate(
            [xt, np.ones((NSH, 1), dtype=np.float32)], axis=1).astype(ph2np)
        xta_dev = np.ascontiguousarray(
            xta.reshape(NT, 128, NAUG).transpose(1, 0, 2).reshape(128, NT * NAUG))
        in_maps.append({"xdn": xdn16, "xta": xta_dev,
                        "cst16": cst16, "cst32": cst32})
    return in_maps


def gather_output(results, codewords):
    E = np.zeros((B, K, D), dtype=np.float32)
    for core, res in enumerate(results):
        full4 = res["eout"].reshape(4, K, NAUG)
        part = full4.sum(axis=0)                      # [K, NAUG]
        E[core // 2] += part[:, 0:D] - part[:, D:D + 1] * codewords
    return E


_NC_CACHE = {}


def _get_nc():
    if "nc" not in _NC_CACHE:
        # Bacc (not plain Bass): its compile() runs the TRN2 sync-wait
        # legalization (max 1 wait per instruction) that walrus requires.
        from concourse import bacc
        nc = build_device_kernel(bacc.Bacc(None))
        if not nc.is_finalized():
            nc.finalize()  # Bacc.finalize = compile (wait legalization) + freeze
        _NC_CACHE["nc"] = nc
    return _NC_CACHE["nc"]


def _install_ntff_hook_shim():
    """Fabricate antenv.axon_hooks if the image lacks it (profiling only)."""
    import sys
    import types
    try:
        from antenv.axon_hooks import get_axon_ntff_profile_hook  # noqa: F401
        return
    except ImportError:
        pass
    from trn_agent_boot.trn_boot import _ntff_profile_via_ctypes
    hook = _ntff_profile_via_ctypes("/opt/axon/libaxon_pjrt.so")
    mod = types.ModuleType("antenv.axon_hooks")
    mod._hook = hook
    mod.get_axon_ntff_profile_hook = lambda: mod._hook
    mod.set_axon_ntff_profile_hook = lambda h: setattr(mod, "_hook", h)
    sys.modules["antenv.axon_hooks"] = mod
    import antenv
    antenv.axon_hooks = mod


def kernel(X, codewords, scale):
    from concourse.bass_utils import run_bass_kernel_spmd

    nc = _get_nc()
    in_maps = make_host_inputs(X, codewords, scale)
    trace = bool(int(os.environ.get("VQ_KERNEL_TRACE", "0")))
    kwargs = {}
    if trace:
        try:
            _install_ntff_hook_shim()
            tmpdir = os.environ.get("VQ_KERNEL_TMPDIR")
            if tmpdir:
                os.makedirs(tmpdir, exist_ok=True)
                kwargs["tmpdir"] = tmpdir
        except Exception as e:  # profiling must never break execution
            print(f"ntff hook install failed: {e}")
            trace = False
    res = run_bass_kernel_spmd(nc, in_maps, core_ids=list(range(NCORES)),
                               trace=trace, **kwargs)
    if trace and res.exec_time_ns is not None:
        print(f"HW exec time: {res.exec_time_ns} ns")
    return gather_output(res.results, np.asarray(codewords, np.float32))
